# revision 13
# baseline (speedup 1.0000x reference)
"""MoE layer (top-2 of 8 experts) — routed Trainium2 Bass kernel.

Data-parallel over tokens across 8 NeuronCores (2048 tokens/core), expert
weights replicated (bf16).  Unlike the dense baseline (all 8 experts for
every token), this kernel routes: tokens are compacted into per-expert
DRAM buffers via dma_scatter_add, each expert's FFN runs only on its own
~512 tokens (capacity 640), and the top-2 combine is a pair of
dma_gather(transpose=True) reads + one DVE add.  4x fewer matmul rows.

Per-core pipeline:
  gate   : psum_g[8,512] = Wg.T @ X.T  (exact fp32) -> top-2 mask per token
  index  : chunk-wise cumsum (Hillis-Steele on DVE) + strict-triangular
           PE matmul -> per-(token,expert) slot; slot lists for the
           scatter/gather sides built with PE transposes + a 16->128
           replication matmul.
  scatter: dma_scatter_add of bf16 x rows into zeroed xc[e*CAP + slot]
           (2 calls: rank-0 / rank-1 slots; collision-free by construction)
  ffn    : per expert: dma_start_transpose loads xc slab feature-major;
           FM (bf16) + GELU(+b1); second matmul is flipped (h-block as
           lhsT) so the output is token(slot)-major -> plain DMA to yc.
  combine: dma_gather(transpose=True) y0T/y1T feature-major by slot list,
           DVE add, then per 128-token block: mask x b2 matmul into PSUM
           followed by accumulating PE transposes; activation copy out.
"""

import os
import sys

sys.path.insert(0, "/opt/trn_rl_repo")

STOP = os.environ.get("K_STOP", "full")   # head|scatter|ffn|gather|full

from contextlib import ExitStack

import numpy as np
import ml_dtypes

import concourse.bacc as bacc
import concourse.bass as bass
import concourse.mybir as mybir
import concourse.tile as tile
from concourse import bass_utils
from concourse.masks import make_identity

N_CORES = 8
B, S, D, E, H = 4, 4096, 256, 8, 512
T = B * S                      # 16384 tokens total
TC = T // N_CORES              # 2048 tokens per core
CAP = 640                      # per-expert token capacity (mean 512, +6.5 sigma)
NCH = 16                       # 128-token chunks per core
DC = D // 128                  # 2
HC = H // 128                  # 4
TRASH = E * CAP                # overflow slot base
NXC = E * CAP + 16             # xc/yc row count (incl. trash rows)

F32 = mybir.dt.float32
F32R = mybir.dt.float32r
BF16 = mybir.dt.bfloat16
I16 = mybir.dt.int16
I32 = mybir.dt.int32
GELU = mybir.ActivationFunctionType.Gelu
IDENT = mybir.ActivationFunctionType.Identity
COPY = mybir.ActivationFunctionType.Copy
ALU = mybir.AluOpType


def _emit(tc: tile.TileContext, ctx: ExitStack, t_in: dict, scratch: dict, t_out):
    nc = tc.nc
    x_d = t_in["x"]
    wg_d, bg_d = t_in["Wg"], t_in["bg"]
    w1_d, b1_d = t_in["W1"], t_in["b1"]
    w2_d, b2_d = t_in["W2"], t_in["b2"]
    xc_d, yc_d = scratch["xc"], scratch["yc"]
    y_d = t_out

    singles = ctx.enter_context(tc.tile_pool(name="singles", bufs=1))
    xpool = ctx.enter_context(tc.tile_pool(name="xpool", bufs=1))
    hdpool = ctx.enter_context(tc.tile_pool(name="hdpool", bufs=1))
    xcpool = ctx.enter_context(tc.tile_pool(name="xcpool", bufs=2))
    hpool = ctx.enter_context(tc.tile_pool(name="hpool", bufs=2))
    ypool = ctx.enter_context(tc.tile_pool(name="ypool", bufs=4))
    opool = ctx.enter_context(tc.tile_pool(name="opool", bufs=1))
    ps_hd = ctx.enter_context(tc.tile_pool(name="ps_hd", bufs=2, space="PSUM"))
    ps_fa = ctx.enter_context(tc.tile_pool(name="ps_fa", bufs=2, space="PSUM"))
    ps_fb = ctx.enter_context(tc.tile_pool(name="ps_fb", bufs=2, space="PSUM"))
    ps_sm = ctx.enter_context(tc.tile_pool(name="ps_sm", bufs=2, space="PSUM"))

    def hd_ps():
        return ps_hd.tile([128, 512], F32, tag="hd", name="hd")

    # ---- constants ------------------------------------------------------
    ident = singles.tile([128, 128], F32)
    make_identity(nc, ident[:])
    ident_bf = singles.tile([128, 128], BF16)
    make_identity(nc, ident_bf[:])
    # strict upper-triangular as stored [k, p]: 1 iff k < p  (so that
    # matmul gives out[p, e] = sum_{k<p} rhs[k, e])
    triu = singles.tile([128, 128], F32)
    nc.gpsimd.memset(triu[:], 1.0)
    # affine iota value = p*1 + f*(-1); keep where iota < 0 (k < p reversed:
    # stored [k(part), p(free)]: value = k - p; keep (k - p) < 0 -> 1 else 0
    nc.gpsimd.affine_select(
        out=triu[:], in_=triu[:], compare_op=ALU.is_gt, fill=0.0,
        base=0, pattern=[[1, 128]], channel_multiplier=-1,
    )
    # 16->128 replication matrix: R16[k, p] = 1 iff p % 16 == k
    r16 = singles.tile([16, 128], F32)
    nc.vector.memset(r16[:], 0.0)
    for g in range(8):
        nc.vector.tensor_copy(r16[:, g * 16:(g + 1) * 16], ident[:16, :16])

    # ---- weights / biases ----------------------------------------------
    w1_sb = singles.tile([128, E, DC, H], BF16)
    w2_sb = singles.tile([128, E, HC, D], BF16)
    for e in range(E // 2):
        nc.scalar.dma_start(out=w1_sb[:, e], in_=w1_d[:, e])
        nc.scalar.dma_start(out=w2_sb[:, e], in_=w2_d[:, e])
    wg_sb = singles.tile([128, DC, E], F32)
    nc.sync.dma_start(out=wg_sb[:], in_=wg_d[:])
    b1_sb = singles.tile([128, E, HC], F32)
    nc.sync.dma_start(out=b1_sb[:], in_=b1_d[:])
    b2_sb = singles.tile([E, D], F32R)
    nc.sync.dma_start(out=b2_sb[:], in_=b2_d[:])
    bg_sb = singles.tile([E, 1], F32)
    nc.sync.dma_start(out=bg_sb[:], in_=bg_d[:, None])

    # ---- zero-fill xc + yc trash rows ----------------------------------
    zt = singles.tile([128, 10, D], BF16)
    nc.vector.memset(zt[:], 0.0)
    for i in range(4):                      # rows [0, 5120) in 4 x 1280
        nc.scalar.dma_start(
            out=xc_d[i * 1280:(i + 1) * 1280, :].rearrange(
                "(p c) d -> p c d", c=10),
            in_=zt[:],
        )
    nc.scalar.dma_start(out=xc_d[TRASH:NXC, :], in_=zt[:16, 0, :])
    nc.scalar.dma_start(out=yc_d[TRASH:NXC, :], in_=zt[:16, 0, :])

    # ---- x loads + transposes + gate, pipelined per 512-token tile -----
    # x_t[p, g] = token g*128 + p   (for gate path)
    x_t = xpool.tile([128, NCH, D], F32, name="x_t")
    xv = x_d.rearrange("(g p) d -> p g d", p=128)
    for s in range(4):
        nc.sync.dma_start(out=x_t[:, s * 4:(s + 1) * 4, :],
                          in_=xv[:, s * 4:(s + 1) * 4, :])
    # x_sc row i at (i%128, i//128) = token lam(i) = (i%16)*128 + i//16
    # row(q,g,w) = q*128 + g*8 + w at partition w*16+q, free g
    x_sc = xpool.tile([128, NCH, D], F32, name="x_sc")
    nc.scalar.dma_start(
        out=x_sc[:], in_=x_d.rearrange("(q g w) d -> w q g d", q=16, g=16, w=8)
    )
    x_bf = xpool.tile([128, NCH, D], BF16, name="x_bf")
    nc.scalar.activation(x_bf[:], x_sc[:], COPY)

    xT = xpool.tile([128, DC, TC], F32, name="xT")
    g_sb = hdpool.tile([E, TC], F32, name="g_sb")
    for s in range(4):
        for g in range(s * 4, (s + 1) * 4):
            for dc in range(DC):
                ps_t = hd_ps()
                nc.tensor.transpose(
                    out=ps_t[:, :128], in_=x_t[:, g, dc * 128:(dc + 1) * 128],
                    identity=ident[:],
                )
                nc.vector.tensor_copy(
                    xT[:, dc, g * 128:(g + 1) * 128], ps_t[:, :128])
        ps_gate = hd_ps()
        for dc in range(DC):
            nc.tensor.matmul(
                ps_gate[:E, :], wg_sb[:, dc, :], xT[:, dc, s * 512:(s + 1) * 512],
                start=(dc == 0), stop=(dc == DC - 1),
            )
        nc.scalar.activation(
            g_sb[:, s * 512:(s + 1) * 512], ps_gate[:E, :], IDENT, bias=bg_sb[:, 0:1]
        )

    # ---- top-2 mask + slot machinery -----------------------------------
    # gtok[:, ch, :] = scores of token ch*128+p
    gtok = hdpool.tile([128, NCH, E], F32, name="gtok")
    for ch in range(NCH):
        ps_t = hd_ps()
        nc.tensor.transpose(
            out=ps_t[:, :E], in_=g_sb[:, ch * 128:(ch + 1) * 128],
            identity=ident[:E, :E],
        )
        nc.vector.tensor_copy(gtok[:, ch, :], ps_t[:, :E])
    m8 = hdpool.tile([128, NCH, 8], F32, name="m8")
    for ch in range(NCH):
        nc.vector.max(m8[:, ch, :], gtok[:, ch, :])
    mask_all = hdpool.tile([128, NCH, E], F32, name="mask_all")
    nc.vector.tensor_tensor(
        out=mask_all[:], in0=gtok[:], in1=m8[:, :, 1:2].to_broadcast([128, NCH, E]),
        op=ALU.is_ge,
    )
    sel0 = hdpool.tile([128, NCH, E], F32, name="sel0")
    nc.vector.tensor_tensor(
        out=sel0[:], in0=gtok[:], in1=m8[:, :, 0:1].to_broadcast([128, NCH, E]),
        op=ALU.is_ge,
    )
    sel1 = hdpool.tile([128, NCH, E], F32, name="sel1")
    nc.vector.tensor_tensor(out=sel1[:], in0=mask_all[:], in1=sel0[:], op=ALU.subtract)

    # inclusive cumsum over chunk axis (Hillis-Steele, ping-pong)
    cs = [hdpool.tile([128, NCH, E], F32, name=f"cs{i}") for i in range(2)]
    src = mask_all
    for i, d in enumerate((1, 2, 4, 8)):
        dst = cs[i % 2]
        nc.vector.tensor_tensor(
            out=dst[:, d:, :], in0=src[:, d:, :], in1=src[:, :NCH - d, :], op=ALU.add
        )
        nc.vector.tensor_copy(dst[:, :d, :], src[:, :d, :])
        src = dst
    csum = src                                   # [128, NCH, E] inclusive

    # partition-exclusive prefix of row totals
    rowtot = hdpool.tile([128, E], F32, name="rowtot")
    nc.vector.tensor_copy(rowtot[:], csum[:, NCH - 1, :])
    ps_pref = hd_ps()
    nc.tensor.matmul(ps_pref[:, :E], triu[:], rowtot[:], start=True, stop=True)

    # base[p, e] = pref[p, e] + e*CAP
    eoff = hdpool.tile([128, E], F32, name="eoff")
    for e in range(E):
        nc.vector.memset(eoff[:, e:e + 1], float(e * CAP))
    base_sb = hdpool.tile([128, E], F32, name="base")
    nc.vector.tensor_tensor(out=base_sb[:], in0=ps_pref[:, :E], in1=eoff[:], op=ALU.add)

    # oslot[t, e] = excl-cumsum + base ; overflow -> TRASH
    excl = hdpool.tile([128, NCH, E], F32, name="excl")
    nc.vector.tensor_tensor(out=excl[:], in0=csum[:], in1=mask_all[:], op=ALU.subtract)
    oslot = hdpool.tile([128, NCH, E], F32, name="oslot")
    nc.vector.tensor_tensor(
        out=oslot[:], in0=excl[:],
        in1=base_sb[:, None, :].to_broadcast([128, NCH, E]), op=ALU.add,
    )
    ov01 = hdpool.tile([128, NCH, E], F32, name="ov01")
    nc.vector.tensor_scalar(
        out=ov01[:], in0=excl[:], scalar1=float(CAP), scalar2=None, op0=ALU.is_ge
    )
    notover = hdpool.tile([128, NCH, E], F32, name="notover")
    nc.vector.tensor_scalar(
        out=notover[:], in0=ov01[:], scalar1=-1.0, scalar2=1.0,
        op0=ALU.mult, op1=ALU.add,
    )
    ovtrash = hdpool.tile([128, NCH, E], F32, name="ovtrash")
    nc.vector.tensor_scalar(
        out=ovtrash[:], in0=ov01[:], scalar1=float(TRASH), scalar2=None,
        op0=ALU.mult,
    )
    nc.vector.tensor_tensor(out=oslot[:], in0=oslot[:], in1=notover[:], op=ALU.mult)
    nc.vector.tensor_tensor(out=oslot[:], in0=oslot[:], in1=ovtrash[:], op=ALU.add)

    # o0/o1[p, ch] = slot of token ch*128+p for its rank-0/1 expert
    o_sb = []
    for r, sel in ((0, sel0), (1, sel1)):
        tmp = hdpool.tile([128, NCH, E], F32, name=f"otmp{r}")
        nc.vector.tensor_tensor(out=tmp[:], in0=sel[:], in1=oslot[:], op=ALU.mult)
        o_r = hdpool.tile([128, NCH], F32, name=f"o{r}")
        nc.vector.tensor_reduce(o_r[:], tmp[:], axis=mybir.AxisListType.X, op=ALU.add)
        o_sb.append(o_r)

    # idx lists (int16, 16-wrapped+replicated): idx_r[i] = o_r[lam(i)]
    idx = []
    for r in range(2):
        ps_t = hd_ps()
        nc.tensor.transpose(out=ps_t[:16, :128], in_=o_sb[r][:], identity=ident[:])
        t_sb = hdpool.tile([16, 128], F32, name=f"t_sb{r}")
        nc.vector.tensor_copy(t_sb[:], ps_t[:16, :128])
        ps_r = hd_ps()
        nc.tensor.matmul(ps_r[:, :128], r16[:], t_sb[:], start=True, stop=True)
        idx_r = hdpool.tile([128, 128], I16, name=f"idx{r}")
        nc.vector.tensor_copy(idx_r[:], ps_r[:, :128])
        idx.append(idx_r)

    # FM gather index lists: slot lists e*CAP + [0..CAP) in 16-wrap layout
    base16 = hdpool.tile([16, CAP // 16], F32, name="base16")
    for f in range(CAP // 16):
        nc.vector.memset(base16[:, f:f + 1], float(f * 16))
    # qcol[q] = q: reduce(ident16 * [16f row values]) / 16
    qtmp = hdpool.tile([16, 16], F32, name="qtmp")
    nc.vector.tensor_tensor(
        out=qtmp[:], in0=ident[:16, :16], in1=base16[:, :16], op=ALU.mult
    )
    qcol = hdpool.tile([16, 1], F32, name="qcol")
    nc.vector.tensor_reduce(qcol[:], qtmp[:], axis=mybir.AxisListType.X, op=ALU.add)
    nc.vector.tensor_scalar(
        out=qcol[:], in0=qcol[:], scalar1=1.0 / 16.0, scalar2=None, op0=ALU.mult
    )
    nc.vector.tensor_tensor(
        out=base16[:], in0=base16[:],
        in1=qcol[:].to_broadcast([16, CAP // 16]), op=ALU.add,
    )
    ps_bi = hd_ps()
    nc.tensor.matmul(ps_bi[:, :CAP // 16], r16[:], base16[:], start=True, stop=True)
    base128 = hdpool.tile([128, CAP // 16], F32, name="base128")
    nc.vector.tensor_copy(base128[:], ps_bi[:, :CAP // 16])
    idx_fm = []
    for e in range(E):
        fme = hdpool.tile([128, CAP // 16], F32, name=f"fme{e}")
        nc.vector.tensor_scalar(
            out=fme[:], in0=base128[:], scalar1=float(e * CAP), scalar2=None,
            op0=ALU.add,
        )
        ie = hdpool.tile([128, CAP // 16], I16, name=f"ifm{e}")
        nc.vector.tensor_copy(ie[:], fme[:])
        idx_fm.append(ie)

    # mt3[e, f, q] = mask[token q*128+f, e]  (lhsT slabs for b2 combine)
    mt3 = hdpool.tile([E, 128, NCH], F32R, name="mt3")
    for ch in range(NCH):
        ps_mt = hd_ps()
        nc.tensor.transpose(
            out=ps_mt[:E, :128], in_=mask_all[:, ch, :], identity=ident[:]
        )
        nc.vector.tensor_copy(mt3[:, :, ch], ps_mt[:E, :128])

    def _debug_out():
        outsb = opool.tile([128, NCH, D], F32, name="outsb_dbg")
        nc.vector.memset(outsb[:], 0.0)
        nc.sync.dma_start(
            out=y_d.rearrange("(q g w) d -> w q g d", q=16, g=16, w=8),
            in_=outsb[:],
        )

    if STOP == "head":
        _debug_out()
        return

    if "dbg_idx" in scratch:
        for r in range(2):
            nc.sync.dma_start(
                out=scratch["dbg_idx"][r * 128:(r + 1) * 128, :], in_=idx[r][:])

    # ---- scatter x rows into xc (512-idx chunks: desc carveout limit) --
    for r in range(2):
        for c in range(4):
            nc.gpsimd.dma_scatter_add(
                out_ap=xc_d[:, :], in_ap=x_bf[:, c * 4:(c + 1) * 4, :],
                idxs_ap=idx[r][:, c * 32:(c + 1) * 32],
                num_idxs=512, num_idxs_reg=512, elem_size=D, queue_num=0,
            )

    if STOP == "scatter":
        _debug_out()
        return

    # ---- per-expert FFN -------------------------------------------------
    for e in range(E):
        if e == 0:
            for e2 in range(E // 2, E):
                nc.sync.dma_start(out=w1_sb[:, e2], in_=w1_d[:, e2])
                nc.sync.dma_start(out=w2_sb[:, e2], in_=w2_d[:, e2])
        xcT = xcpool.tile([128, DC, CAP], BF16, tag="xcT", name=f"xcT{e}")
        nc.gpsimd.dma_gather(
            out_ap=xcT[:], in_ap=xc_d[:, :], idxs_ap=idx_fm[e][:],
            num_idxs=CAP, num_idxs_reg=CAP, elem_size=D, transpose=True,
            queue_num=0,
        )

        h_sb = hpool.tile([128, HC, CAP], BF16, tag="h", name=f"h{e}")
        for hc in range(HC):
            ps_ha = ps_fa.tile([128, 512], F32, tag="fmA")
            ps_hb = ps_fb.tile([128, 128], F32, tag="fmB")
            for dc in range(DC):
                nc.tensor.matmul(
                    ps_ha[:], w1_sb[:, e, dc, hc * 128:(hc + 1) * 128],
                    xcT[:, dc, 0:512], start=(dc == 0), stop=(dc == DC - 1),
                )
            for dc in range(DC):
                nc.tensor.matmul(
                    ps_hb[:], w1_sb[:, e, dc, hc * 128:(hc + 1) * 128],
                    xcT[:, dc, 512:CAP], start=(dc == 0), stop=(dc == DC - 1),
                )
            nc.scalar.activation(
                h_sb[:, hc, 0:512], ps_ha[:], GELU, bias=b1_sb[:, e, hc:hc + 1]
            )
            nc.scalar.activation(
                h_sb[:, hc, 512:CAP], ps_hb[:], GELU, bias=b1_sb[:, e, hc:hc + 1]
            )
        # flipped second matmul: out[token(slot), d]
        for tb in range(CAP // 128):
            ps_o = ps_sm.tile([128, D], F32, tag="sm")
            for hc in range(HC):
                nc.tensor.matmul(
                    ps_o[:], h_sb[:, hc, tb * 128:(tb + 1) * 128],
                    w2_sb[:, e, hc, :], start=(hc == 0), stop=(hc == HC - 1),
                )
            yst = ypool.tile([128, D], BF16, tag="yst", name=f"yst{e}_{tb}")
            nc.scalar.activation(yst[:], ps_o[:], COPY)
            eng = nc.sync if (e * 5 + tb) % 2 == 0 else nc.scalar
            eng.dma_start(
                out=yc_d[e * CAP + tb * 128:e * CAP + (tb + 1) * 128, :],
                in_=yst[:],
            )

    if STOP == "ffn":
        _debug_out()
        return

    # ---- combine --------------------------------------------------------
    yT = [[None] * 4, [None] * 4]
    for c in range(4):
        for r in range(2):
            y_rc = opool.tile([128, DC, 512], BF16, tag=f"yT{r}_{c}",
                              name=f"yT{r}_{c}")
            nc.gpsimd.dma_gather(
                out_ap=y_rc[:], in_ap=yc_d[:, :],
                idxs_ap=idx[r][:, c * 32:(c + 1) * 32],
                num_idxs=512, num_idxs_reg=512, elem_size=D, transpose=True,
                queue_num=0,
            )
            yT[r][c] = y_rc
    if STOP == "gather":
        _debug_out()
        return

    ysum = opool.tile([128, DC, TC], F32, name="ysum")
    outsb = opool.tile([128, NCH, D], F32, name="outsb")
    for c in range(4):
        nc.vector.tensor_tensor(
            out=ysum[:, :, c * 512:(c + 1) * 512], in0=yT[0][c][:],
            in1=yT[1][c][:], op=ALU.add,
        )
        for fb in range(c * 4, (c + 1) * 4):
            ps_o = ps_sm.tile([128, D], F32, tag="sm")
            nc.tensor.matmul(
                ps_o[:], mt3[:, fb * 8:(fb + 1) * 8, :], b2_sb[:],
                start=True, stop=False, skip_group_check=True,
            )
            for dc in range(DC):
                nc.tensor.matmul(
                    ps_o[:, dc * 128:(dc + 1) * 128],
                    ysum[:, dc, fb * 128:(fb + 1) * 128], ident[:],
                    is_transpose=True, start=False, stop=(dc == DC - 1),
                    skip_group_check=True,
                )
            nc.scalar.activation(outsb[:, fb, :], ps_o[:], COPY)
    nc.sync.dma_start(
        out=y_d.rearrange("(q g w) d -> w q g d", q=16, g=16, w=8),
        in_=outsb[:],
    )


_CACHE = {}


def _build():
    if "nc" in _CACHE:
        return _CACHE["nc"]
    nc = bacc.Bacc("TRN2", target_bir_lowering=False)
    t_in = {
        "x": nc.dram_tensor("x", [TC, D], F32, kind="ExternalInput"),
        "Wg": nc.dram_tensor("Wg", [128, DC, E], F32, kind="ExternalInput"),
        "bg": nc.dram_tensor("bg", [E], F32, kind="ExternalInput"),
        "W1": nc.dram_tensor("W1", [128, E, DC, H], BF16, kind="ExternalInput"),
        "b1": nc.dram_tensor("b1", [128, E, HC], F32, kind="ExternalInput"),
        "W2": nc.dram_tensor("W2", [128, E, HC, D], BF16, kind="ExternalInput"),
        "b2": nc.dram_tensor("b2", [E, D], F32R, kind="ExternalInput"),
    }
    dbg = os.environ.get("K_DEBUG") == "1"
    # NOTE: scratch must be ExternalOutput, not Internal: Internal DRAM
    # compiles to a fixed NEFF address, and the 8 SPMD cores share HBM --
    # every core would scatter into the same physical buffer.  External
    # outputs get per-core runtime allocations.
    scratch = {
        "xc": nc.dram_tensor("xc", [NXC, D], BF16, kind="ExternalOutput"),
        "yc": nc.dram_tensor("yc", [NXC, D], BF16, kind="ExternalOutput"),
    }
    if dbg:
        scratch["dbg_idx"] = nc.dram_tensor(
            "dbg_idx", [256, 128], mybir.dt.int16, kind="ExternalOutput")
    y_d = nc.dram_tensor("y", [TC, D], F32, kind="ExternalOutput")
    with tile.TileContext(nc) as tc:
        with ExitStack() as ctx:
            _emit(tc, ctx, t_in, scratch, y_d)
    nc.compile()
    _CACHE["nc"] = nc
    return nc


def _prep_shared(inputs):
    f = lambda a: np.ascontiguousarray(np.asarray(a, dtype=np.float32))
    bf = lambda a: np.ascontiguousarray(a.astype(ml_dtypes.bfloat16))
    w1 = f(inputs["W1"])
    w2 = f(inputs["W2"])
    wg = f(inputs["Wg"])
    b1 = f(inputs["b1"])
    return {
        "Wg": np.ascontiguousarray(wg.reshape(DC, 128, E).transpose(1, 0, 2)),
        "bg": f(inputs["bg"]),
        "W1": bf(w1.reshape(E, DC, 128, H).transpose(2, 0, 1, 3)),
        "b1": np.ascontiguousarray(b1.reshape(E, HC, 128).transpose(2, 0, 1)),
        "W2": bf(w2.reshape(E, HC, 128, D).transpose(2, 0, 1, 3)),
        "b2": f(inputs["b2"]),
    }


def _run(inputs: dict, trace: bool = False, **kw):
    nc = _build()
    x = np.ascontiguousarray(np.asarray(inputs["x"], dtype=np.float32)).reshape(T, D)
    shared = _prep_shared(inputs)
    in_maps = [
        {"x": x[c * TC:(c + 1) * TC], **shared} for c in range(N_CORES)
    ]
    br = bass_utils.run_bass_kernel_spmd(
        nc, in_maps, core_ids=list(range(N_CORES)), trace=trace, **kw
    )
    out = np.concatenate([r["y"] for r in br.results], axis=0)
    return out.reshape(B, S, D), br


def kernel(**inputs) -> np.ndarray:
    out, _ = _run(inputs, trace=False)
    return out


# revision 14
# speedup vs baseline: 1.0139x; 1.0139x over previous
"""MoE layer (top-2 of 8 experts) — routed Trainium2 Bass kernel.

Data-parallel over tokens across 8 NeuronCores (2048 tokens/core), expert
weights replicated (bf16).  Unlike the dense baseline (all 8 experts for
every token), this kernel routes: tokens are compacted into per-expert
DRAM buffers via dma_scatter_add, each expert's FFN runs only on its own
~512 tokens (capacity 640), and the top-2 combine is a pair of
dma_gather(transpose=True) reads + one DVE add.  4x fewer matmul rows.

Per-core pipeline:
  gate   : psum_g[8,512] = Wg.T @ X.T  (exact fp32) -> top-2 mask per token
  index  : chunk-wise cumsum (Hillis-Steele on DVE) + strict-triangular
           PE matmul -> per-(token,expert) slot; slot lists for the
           scatter/gather sides built with PE transposes + a 16->128
           replication matmul.
  scatter: dma_scatter_add of bf16 x rows into zeroed xc[e*CAP + slot]
           (2 calls: rank-0 / rank-1 slots; collision-free by construction)
  ffn    : per expert: dma_start_transpose loads xc slab feature-major;
           FM (bf16) + GELU(+b1); second matmul is flipped (h-block as
           lhsT) so the output is token(slot)-major -> plain DMA to yc.
  combine: dma_gather(transpose=True) y0T/y1T feature-major by slot list,
           DVE add, then per 128-token block: mask x b2 matmul into PSUM
           followed by accumulating PE transposes; activation copy out.
"""

import os
import sys

sys.path.insert(0, "/opt/trn_rl_repo")

STOP = os.environ.get("K_STOP", "full")   # head|scatter|ffn|gather|full

from contextlib import ExitStack

import numpy as np
import ml_dtypes

import concourse.bacc as bacc
import concourse.bass as bass
import concourse.mybir as mybir
import concourse.tile as tile
from concourse import bass_utils
from concourse.masks import make_identity

N_CORES = 8
B, S, D, E, H = 4, 4096, 256, 8, 512
T = B * S                      # 16384 tokens total
TC = T // N_CORES              # 2048 tokens per core
CAP = 640                      # per-expert token capacity (mean 512, +6.5 sigma)
NCH = 16                       # 128-token chunks per core
DC = D // 128                  # 2
HC = H // 128                  # 4
TRASH = E * CAP                # overflow slot base
NXC = E * CAP + 16             # xc/yc row count (incl. trash rows)

F32 = mybir.dt.float32
F32R = mybir.dt.float32r
BF16 = mybir.dt.bfloat16
I16 = mybir.dt.int16
I32 = mybir.dt.int32
GELU = mybir.ActivationFunctionType.Gelu
IDENT = mybir.ActivationFunctionType.Identity
COPY = mybir.ActivationFunctionType.Copy
ALU = mybir.AluOpType


def _emit(tc: tile.TileContext, ctx: ExitStack, t_in: dict, scratch: dict, t_out):
    nc = tc.nc
    x_d = t_in["x"]
    wg_d, bg_d = t_in["Wg"], t_in["bg"]
    w1_d, b1_d = t_in["W1"], t_in["b1"]
    w2_d, b2_d = t_in["W2"], t_in["b2"]
    xc_d, yc_d = scratch["xc"], scratch["yc"]
    y_d = t_out

    singles = ctx.enter_context(tc.tile_pool(name="singles", bufs=1))
    xpool = ctx.enter_context(tc.tile_pool(name="xpool", bufs=1))
    hdpool = ctx.enter_context(tc.tile_pool(name="hdpool", bufs=1))
    xcpool = ctx.enter_context(tc.tile_pool(name="xcpool", bufs=2))
    hpool = ctx.enter_context(tc.tile_pool(name="hpool", bufs=2))
    ypool = ctx.enter_context(tc.tile_pool(name="ypool", bufs=4))
    opool = ctx.enter_context(tc.tile_pool(name="opool", bufs=1))
    ps_hd = ctx.enter_context(tc.tile_pool(name="ps_hd", bufs=2, space="PSUM"))
    ps_fa = ctx.enter_context(tc.tile_pool(name="ps_fa", bufs=2, space="PSUM"))
    ps_fb = ctx.enter_context(tc.tile_pool(name="ps_fb", bufs=2, space="PSUM"))
    ps_sm = ctx.enter_context(tc.tile_pool(name="ps_sm", bufs=2, space="PSUM"))

    def hd_ps():
        return ps_hd.tile([128, 512], F32, tag="hd", name="hd")

    # ---- constants ------------------------------------------------------
    ident = singles.tile([128, 128], F32)
    make_identity(nc, ident[:])
    ident_bf = singles.tile([128, 128], BF16)
    make_identity(nc, ident_bf[:])
    # strict upper-triangular as stored [k, p]: 1 iff k < p  (so that
    # matmul gives out[p, e] = sum_{k<p} rhs[k, e])
    triu = singles.tile([128, 128], F32)
    nc.gpsimd.memset(triu[:], 1.0)
    # affine iota value = p*1 + f*(-1); keep where iota < 0 (k < p reversed:
    # stored [k(part), p(free)]: value = k - p; keep (k - p) < 0 -> 1 else 0
    nc.gpsimd.affine_select(
        out=triu[:], in_=triu[:], compare_op=ALU.is_gt, fill=0.0,
        base=0, pattern=[[1, 128]], channel_multiplier=-1,
    )
    # 16->128 replication matrix: R16[k, p] = 1 iff p % 16 == k
    r16 = singles.tile([16, 128], F32)
    nc.vector.memset(r16[:], 0.0)
    for g in range(8):
        nc.vector.tensor_copy(r16[:, g * 16:(g + 1) * 16], ident[:16, :16])

    # ---- weights / biases ----------------------------------------------
    w1_sb = singles.tile([128, E, DC, H], BF16)
    w2_sb = singles.tile([128, E, HC, D], BF16)
    for e in range(E // 2):
        nc.scalar.dma_start(out=w1_sb[:, e], in_=w1_d[:, e])
        nc.scalar.dma_start(out=w2_sb[:, e], in_=w2_d[:, e])
    wg_sb = singles.tile([128, DC, E], F32)
    nc.sync.dma_start(out=wg_sb[:], in_=wg_d[:])
    b1_sb = singles.tile([128, E, HC], F32)
    nc.sync.dma_start(out=b1_sb[:], in_=b1_d[:])
    b2_sb = singles.tile([E, D], F32R)
    nc.sync.dma_start(out=b2_sb[:], in_=b2_d[:])
    bg_sb = singles.tile([E, 1], F32)
    nc.sync.dma_start(out=bg_sb[:], in_=bg_d[:, None])

    # ---- zero-fill xc + yc trash rows ----------------------------------
    zt = singles.tile([128, 10, D], BF16)
    nc.vector.memset(zt[:], 0.0)
    for i in range(4):                      # rows [0, 5120) in 4 x 1280
        nc.scalar.dma_start(
            out=xc_d[i * 1280:(i + 1) * 1280, :].rearrange(
                "(p c) d -> p c d", c=10),
            in_=zt[:],
        )
    nc.scalar.dma_start(out=xc_d[TRASH:NXC, :], in_=zt[:16, 0, :])
    nc.scalar.dma_start(out=yc_d[TRASH:NXC, :], in_=zt[:16, 0, :])

    # ---- x loads + transposes + gate, pipelined per 512-token tile -----
    # x_t[p, g] = token g*128 + p   (for gate path)
    x_t = xpool.tile([128, NCH, D], F32, name="x_t")
    xv = x_d.rearrange("(g p) d -> p g d", p=128)
    for s in range(4):
        nc.sync.dma_start(out=x_t[:, s * 4:(s + 1) * 4, :],
                          in_=xv[:, s * 4:(s + 1) * 4, :])
    # x_sc row i at (i%128, i//128) = token lam(i) = (i%16)*128 + i//16
    # row(q,g,w) = q*128 + g*8 + w at partition w*16+q, free g
    x_sc = xpool.tile([128, NCH, D], F32, name="x_sc")
    nc.scalar.dma_start(
        out=x_sc[:], in_=x_d.rearrange("(q g w) d -> w q g d", q=16, g=16, w=8)
    )
    x_bf = xpool.tile([128, NCH, D], BF16, name="x_bf")
    nc.scalar.activation(x_bf[:], x_sc[:], COPY)

    xT = xpool.tile([128, DC, TC], F32, name="xT")
    g_sb = hdpool.tile([E, TC], F32, name="g_sb")
    for s in range(4):
        for g in range(s * 4, (s + 1) * 4):
            for dc in range(DC):
                ps_t = hd_ps()
                nc.tensor.transpose(
                    out=ps_t[:, :128], in_=x_t[:, g, dc * 128:(dc + 1) * 128],
                    identity=ident[:],
                )
                nc.vector.tensor_copy(
                    xT[:, dc, g * 128:(g + 1) * 128], ps_t[:, :128])
        ps_gate = hd_ps()
        for dc in range(DC):
            nc.tensor.matmul(
                ps_gate[:E, :], wg_sb[:, dc, :], xT[:, dc, s * 512:(s + 1) * 512],
                start=(dc == 0), stop=(dc == DC - 1),
            )
        nc.scalar.activation(
            g_sb[:, s * 512:(s + 1) * 512], ps_gate[:E, :], IDENT, bias=bg_sb[:, 0:1]
        )

    # ---- top-2 mask + slot machinery -----------------------------------
    # gtok[:, ch, :] = scores of token ch*128+p
    gtok = hdpool.tile([128, NCH, E], F32, name="gtok")
    for ch in range(NCH):
        ps_t = hd_ps()
        nc.tensor.transpose(
            out=ps_t[:, :E], in_=g_sb[:, ch * 128:(ch + 1) * 128],
            identity=ident[:E, :E],
        )
        nc.vector.tensor_copy(gtok[:, ch, :], ps_t[:, :E])
    m8 = hdpool.tile([128, NCH, 8], F32, name="m8")
    for ch in range(NCH):
        nc.vector.max(m8[:, ch, :], gtok[:, ch, :])
    mask_all = hdpool.tile([128, NCH, E], F32, name="mask_all")
    nc.vector.tensor_tensor(
        out=mask_all[:], in0=gtok[:], in1=m8[:, :, 1:2].to_broadcast([128, NCH, E]),
        op=ALU.is_ge,
    )
    sel0 = hdpool.tile([128, NCH, E], F32, name="sel0")
    nc.vector.tensor_tensor(
        out=sel0[:], in0=gtok[:], in1=m8[:, :, 0:1].to_broadcast([128, NCH, E]),
        op=ALU.is_ge,
    )
    sel1 = hdpool.tile([128, NCH, E], F32, name="sel1")
    nc.vector.tensor_tensor(out=sel1[:], in0=mask_all[:], in1=sel0[:], op=ALU.subtract)

    # inclusive cumsum over chunk axis (Hillis-Steele, ping-pong)
    cs = [hdpool.tile([128, NCH, E], F32, name=f"cs{i}") for i in range(2)]
    src = mask_all
    for i, d in enumerate((1, 2, 4, 8)):
        dst = cs[i % 2]
        nc.vector.tensor_tensor(
            out=dst[:, d:, :], in0=src[:, d:, :], in1=src[:, :NCH - d, :], op=ALU.add
        )
        nc.vector.tensor_copy(dst[:, :d, :], src[:, :d, :])
        src = dst
    csum = src                                   # [128, NCH, E] inclusive

    # partition-exclusive prefix of row totals
    rowtot = hdpool.tile([128, E], F32, name="rowtot")
    nc.vector.tensor_copy(rowtot[:], csum[:, NCH - 1, :])
    ps_pref = hd_ps()
    nc.tensor.matmul(ps_pref[:, :E], triu[:], rowtot[:], start=True, stop=True)

    # base[p, e] = pref[p, e] + e*CAP
    eoff = hdpool.tile([128, E], F32, name="eoff")
    for e in range(E):
        nc.vector.memset(eoff[:, e:e + 1], float(e * CAP))
    base_sb = hdpool.tile([128, E], F32, name="base")
    nc.vector.tensor_tensor(out=base_sb[:], in0=ps_pref[:, :E], in1=eoff[:], op=ALU.add)

    # oslot[t, e] = excl-cumsum + base ; overflow -> TRASH
    excl = hdpool.tile([128, NCH, E], F32, name="excl")
    nc.vector.tensor_tensor(out=excl[:], in0=csum[:], in1=mask_all[:], op=ALU.subtract)
    oslot = hdpool.tile([128, NCH, E], F32, name="oslot")
    nc.vector.tensor_tensor(
        out=oslot[:], in0=excl[:],
        in1=base_sb[:, None, :].to_broadcast([128, NCH, E]), op=ALU.add,
    )
    ov01 = hdpool.tile([128, NCH, E], F32, name="ov01")
    nc.vector.tensor_scalar(
        out=ov01[:], in0=excl[:], scalar1=float(CAP), scalar2=None, op0=ALU.is_ge
    )
    notover = hdpool.tile([128, NCH, E], F32, name="notover")
    nc.vector.tensor_scalar(
        out=notover[:], in0=ov01[:], scalar1=-1.0, scalar2=1.0,
        op0=ALU.mult, op1=ALU.add,
    )
    ovtrash = hdpool.tile([128, NCH, E], F32, name="ovtrash")
    nc.vector.tensor_scalar(
        out=ovtrash[:], in0=ov01[:], scalar1=float(TRASH), scalar2=None,
        op0=ALU.mult,
    )
    nc.vector.tensor_tensor(out=oslot[:], in0=oslot[:], in1=notover[:], op=ALU.mult)
    nc.vector.tensor_tensor(out=oslot[:], in0=oslot[:], in1=ovtrash[:], op=ALU.add)

    # per rank: slot list -> idx list -> scatter immediately (one 2048-desc
    # call per rank; descs proven safe up to 2048 for scatter)
    idx = []
    for r, sel in ((0, sel0), (1, sel1)):
        tmp = hdpool.tile([128, NCH, E], F32, name=f"otmp{r}")
        nc.vector.tensor_tensor(out=tmp[:], in0=sel[:], in1=oslot[:], op=ALU.mult)
        o_r = hdpool.tile([128, NCH], F32, name=f"o{r}")
        nc.vector.tensor_reduce(o_r[:], tmp[:], axis=mybir.AxisListType.X, op=ALU.add)
        ps_t = hd_ps()
        nc.tensor.transpose(out=ps_t[:16, :128], in_=o_r[:], identity=ident[:])
        t_sb = hdpool.tile([16, 128], F32, name=f"t_sb{r}")
        nc.vector.tensor_copy(t_sb[:], ps_t[:16, :128])
        ps_r = hd_ps()
        nc.tensor.matmul(ps_r[:, :128], r16[:], t_sb[:], start=True, stop=True)
        idx_r = hdpool.tile([128, 128], I16, name=f"idx{r}")
        nc.vector.tensor_copy(idx_r[:], ps_r[:, :128])
        idx.append(idx_r)
        nc.gpsimd.dma_scatter_add(
            out_ap=xc_d[:, :], in_ap=x_bf[:], idxs_ap=idx_r[:],
            num_idxs=TC, num_idxs_reg=TC, elem_size=D, queue_num=0,
        )

    # FM gather index lists: slot lists e*CAP + [0..CAP) in 16-wrap layout
    base16 = hdpool.tile([16, CAP // 16], F32, name="base16")
    for f in range(CAP // 16):
        nc.vector.memset(base16[:, f:f + 1], float(f * 16))
    # qcol[q] = q: reduce(ident16 * [16f row values]) / 16
    qtmp = hdpool.tile([16, 16], F32, name="qtmp")
    nc.vector.tensor_tensor(
        out=qtmp[:], in0=ident[:16, :16], in1=base16[:, :16], op=ALU.mult
    )
    qcol = hdpool.tile([16, 1], F32, name="qcol")
    nc.vector.tensor_reduce(qcol[:], qtmp[:], axis=mybir.AxisListType.X, op=ALU.add)
    nc.vector.tensor_scalar(
        out=qcol[:], in0=qcol[:], scalar1=1.0 / 16.0, scalar2=None, op0=ALU.mult
    )
    nc.vector.tensor_tensor(
        out=base16[:], in0=base16[:],
        in1=qcol[:].to_broadcast([16, CAP // 16]), op=ALU.add,
    )
    ps_bi = hd_ps()
    nc.tensor.matmul(ps_bi[:, :CAP // 16], r16[:], base16[:], start=True, stop=True)
    base128 = hdpool.tile([128, CAP // 16], F32, name="base128")
    nc.vector.tensor_copy(base128[:], ps_bi[:, :CAP // 16])
    idx_fm = []
    for e in range(E):
        fme = hdpool.tile([128, CAP // 16], F32, name=f"fme{e}")
        nc.vector.tensor_scalar(
            out=fme[:], in0=base128[:], scalar1=float(e * CAP), scalar2=None,
            op0=ALU.add,
        )
        ie = hdpool.tile([128, CAP // 16], I16, name=f"ifm{e}")
        nc.vector.tensor_copy(ie[:], fme[:])
        idx_fm.append(ie)

    # mt3[e, f, q] = mask[token q*128+f, e]; b2tok = mask x b2 token-major,
    # precomputed here (PE has slack) to keep the combine tail lean.
    mt3 = hdpool.tile([E, 128, NCH], F32R, name="mt3")
    for ch in range(NCH):
        ps_mt = hd_ps()
        nc.tensor.transpose(
            out=ps_mt[:E, :128], in_=mask_all[:, ch, :], identity=ident[:]
        )
        nc.vector.tensor_copy(mt3[:, :, ch], ps_mt[:E, :128])
    b2tok = hdpool.tile([128, NCH, D], F32, name="b2tok")
    for fb in range(NCH):
        ps_b2 = hd_ps()
        nc.tensor.matmul(
            ps_b2[:, :D], mt3[:, fb * 8:(fb + 1) * 8, :], b2_sb[:],
            start=True, stop=True,
        )
        nc.scalar.activation(b2tok[:, fb, :], ps_b2[:, :D], COPY)

    def _debug_out():
        outsb = opool.tile([128, NCH, D], F32, name="outsb_dbg")
        nc.vector.memset(outsb[:], 0.0)
        nc.sync.dma_start(
            out=y_d.rearrange("(q g w) d -> w q g d", q=16, g=16, w=8),
            in_=outsb[:],
        )

    if STOP == "head":
        _debug_out()
        return

    if "dbg_idx" in scratch:
        for r in range(2):
            nc.sync.dma_start(
                out=scratch["dbg_idx"][r * 128:(r + 1) * 128, :], in_=idx[r][:])

    if STOP == "scatter":
        _debug_out()
        return

    # ---- per-expert FFN -------------------------------------------------
    for e in range(E):
        if e == 0:
            for e2 in range(E // 2, E):
                nc.sync.dma_start(out=w1_sb[:, e2], in_=w1_d[:, e2])
                nc.sync.dma_start(out=w2_sb[:, e2], in_=w2_d[:, e2])
        xcT = xcpool.tile([128, DC, CAP], BF16, tag="xcT", name=f"xcT{e}")
        nc.gpsimd.dma_gather(
            out_ap=xcT[:], in_ap=xc_d[:, :], idxs_ap=idx_fm[e][:],
            num_idxs=CAP, num_idxs_reg=CAP, elem_size=D, transpose=True,
            queue_num=0,
        )

        h_sb = hpool.tile([128, HC, CAP], BF16, tag="h", name=f"h{e}")
        for hc in range(HC):
            ps_ha = ps_fa.tile([128, 512], F32, tag="fmA")
            ps_hb = ps_fb.tile([128, 128], F32, tag="fmB")
            for dc in range(DC):
                nc.tensor.matmul(
                    ps_ha[:], w1_sb[:, e, dc, hc * 128:(hc + 1) * 128],
                    xcT[:, dc, 0:512], start=(dc == 0), stop=(dc == DC - 1),
                )
            for dc in range(DC):
                nc.tensor.matmul(
                    ps_hb[:], w1_sb[:, e, dc, hc * 128:(hc + 1) * 128],
                    xcT[:, dc, 512:CAP], start=(dc == 0), stop=(dc == DC - 1),
                )
            nc.scalar.activation(
                h_sb[:, hc, 0:512], ps_ha[:], GELU, bias=b1_sb[:, e, hc:hc + 1]
            )
            nc.scalar.activation(
                h_sb[:, hc, 512:CAP], ps_hb[:], GELU, bias=b1_sb[:, e, hc:hc + 1]
            )
        # flipped second matmul: out[token(slot), d]
        for tb in range(CAP // 128):
            ps_o = ps_sm.tile([128, D], F32, tag="sm")
            for hc in range(HC):
                nc.tensor.matmul(
                    ps_o[:], h_sb[:, hc, tb * 128:(tb + 1) * 128],
                    w2_sb[:, e, hc, :], start=(hc == 0), stop=(hc == HC - 1),
                )
            yst = ypool.tile([128, D], BF16, tag="yst", name=f"yst{e}_{tb}")
            nc.scalar.activation(yst[:], ps_o[:], COPY)
            eng = nc.sync if (e * 5 + tb) % 2 == 0 else nc.scalar
            eng.dma_start(
                out=yc_d[e * CAP + tb * 128:e * CAP + (tb + 1) * 128, :],
                in_=yst[:],
            )

    if STOP == "ffn":
        _debug_out()
        return

    # ---- combine --------------------------------------------------------
    yT = [[None] * 4, [None] * 4]
    for c in range(4):
        for r in range(2):
            y_rc = opool.tile([128, DC, 512], BF16, tag=f"yT{r}_{c}",
                              name=f"yT{r}_{c}")
            nc.gpsimd.dma_gather(
                out_ap=y_rc[:], in_ap=yc_d[:, :],
                idxs_ap=idx[r][:, c * 32:(c + 1) * 32],
                num_idxs=512, num_idxs_reg=512, elem_size=D, transpose=True,
                queue_num=0,
            )
            yT[r][c] = y_rc
    if STOP == "gather":
        _debug_out()
        return

    ysum = opool.tile([128, DC, TC], F32, name="ysum")
    outsb = opool.tile([128, NCH, D], F32, name="outsb")
    for c in range(4):
        nc.vector.tensor_tensor(
            out=ysum[:, :, c * 512:(c + 1) * 512], in0=yT[0][c][:],
            in1=yT[1][c][:], op=ALU.add,
        )
        for fb in range(c * 4, (c + 1) * 4):
            ps_o = ps_sm.tile([128, D], F32, tag="sm")
            for dc in range(DC):
                nc.tensor.matmul(
                    ps_o[:, dc * 128:(dc + 1) * 128],
                    ysum[:, dc, fb * 128:(fb + 1) * 128], ident[:],
                    is_transpose=True, start=(dc == 0), stop=(dc == DC - 1),
                    skip_group_check=True,
                )
            nc.vector.tensor_tensor(
                out=outsb[:, fb, :], in0=ps_o[:], in1=b2tok[:, fb, :], op=ALU.add
            )
    nc.sync.dma_start(
        out=y_d.rearrange("(q g w) d -> w q g d", q=16, g=16, w=8),
        in_=outsb[:],
    )


_CACHE = {}


def _build():
    if "nc" in _CACHE:
        return _CACHE["nc"]
    nc = bacc.Bacc("TRN2", target_bir_lowering=False)
    t_in = {
        "x": nc.dram_tensor("x", [TC, D], F32, kind="ExternalInput"),
        "Wg": nc.dram_tensor("Wg", [128, DC, E], F32, kind="ExternalInput"),
        "bg": nc.dram_tensor("bg", [E], F32, kind="ExternalInput"),
        "W1": nc.dram_tensor("W1", [128, E, DC, H], BF16, kind="ExternalInput"),
        "b1": nc.dram_tensor("b1", [128, E, HC], F32, kind="ExternalInput"),
        "W2": nc.dram_tensor("W2", [128, E, HC, D], BF16, kind="ExternalInput"),
        "b2": nc.dram_tensor("b2", [E, D], F32R, kind="ExternalInput"),
    }
    dbg = os.environ.get("K_DEBUG") == "1"
    # NOTE: scratch must be ExternalOutput, not Internal: Internal DRAM
    # compiles to a fixed NEFF address, and the 8 SPMD cores share HBM --
    # every core would scatter into the same physical buffer.  External
    # outputs get per-core runtime allocations.
    scratch = {
        "xc": nc.dram_tensor("xc", [NXC, D], BF16, kind="ExternalOutput"),
        "yc": nc.dram_tensor("yc", [NXC, D], BF16, kind="ExternalOutput"),
    }
    if dbg:
        scratch["dbg_idx"] = nc.dram_tensor(
            "dbg_idx", [256, 128], mybir.dt.int16, kind="ExternalOutput")
    y_d = nc.dram_tensor("y", [TC, D], F32, kind="ExternalOutput")
    with tile.TileContext(nc) as tc:
        with ExitStack() as ctx:
            _emit(tc, ctx, t_in, scratch, y_d)
    nc.compile()
    _CACHE["nc"] = nc
    return nc


def _prep_shared(inputs):
    f = lambda a: np.ascontiguousarray(np.asarray(a, dtype=np.float32))
    bf = lambda a: np.ascontiguousarray(a.astype(ml_dtypes.bfloat16))
    w1 = f(inputs["W1"])
    w2 = f(inputs["W2"])
    wg = f(inputs["Wg"])
    b1 = f(inputs["b1"])
    return {
        "Wg": np.ascontiguousarray(wg.reshape(DC, 128, E).transpose(1, 0, 2)),
        "bg": f(inputs["bg"]),
        "W1": bf(w1.reshape(E, DC, 128, H).transpose(2, 0, 1, 3)),
        "b1": np.ascontiguousarray(b1.reshape(E, HC, 128).transpose(2, 0, 1)),
        "W2": bf(w2.reshape(E, HC, 128, D).transpose(2, 0, 1, 3)),
        "b2": f(inputs["b2"]),
    }


def _run(inputs: dict, trace: bool = False, **kw):
    nc = _build()
    x = np.ascontiguousarray(np.asarray(inputs["x"], dtype=np.float32)).reshape(T, D)
    shared = _prep_shared(inputs)
    in_maps = [
        {"x": x[c * TC:(c + 1) * TC], **shared} for c in range(N_CORES)
    ]
    br = bass_utils.run_bass_kernel_spmd(
        nc, in_maps, core_ids=list(range(N_CORES)), trace=trace, **kw
    )
    out = np.concatenate([r["y"] for r in br.results], axis=0)
    return out.reshape(B, S, D), br


def kernel(**inputs) -> np.ndarray:
    out, _ = _run(inputs, trace=False)
    return out


# revision 15
# speedup vs baseline: 1.0203x; 1.0063x over previous
"""MoE layer (top-2 of 8 experts) — routed Trainium2 Bass kernel.

Data-parallel over tokens across 8 NeuronCores (2048 tokens/core), expert
weights replicated (bf16).  Unlike the dense baseline (all 8 experts for
every token), this kernel routes: tokens are compacted into per-expert
DRAM buffers via dma_scatter_add, each expert's FFN runs only on its own
~512 tokens (capacity 640), and the top-2 combine is a pair of
dma_gather(transpose=True) reads + one DVE add.  4x fewer matmul rows.

Per-core pipeline:
  gate   : psum_g[8,512] = Wg.T @ X.T  (exact fp32) -> top-2 mask per token
  index  : chunk-wise cumsum (Hillis-Steele on DVE) + strict-triangular
           PE matmul -> per-(token,expert) slot; slot lists for the
           scatter/gather sides built with PE transposes + a 16->128
           replication matmul.
  scatter: dma_scatter_add of bf16 x rows into zeroed xc[e*CAP + slot]
           (2 calls: rank-0 / rank-1 slots; collision-free by construction)
  ffn    : per expert: dma_start_transpose loads xc slab feature-major;
           FM (bf16) + GELU(+b1); second matmul is flipped (h-block as
           lhsT) so the output is token(slot)-major -> plain DMA to yc.
  combine: dma_gather(transpose=True) y0T/y1T feature-major by slot list,
           DVE add, then per 128-token block: mask x b2 matmul into PSUM
           followed by accumulating PE transposes; activation copy out.
"""

import os
import sys

sys.path.insert(0, "/opt/trn_rl_repo")

STOP = os.environ.get("K_STOP", "full")   # head|scatter|ffn|gather|full

from contextlib import ExitStack

import numpy as np
import ml_dtypes

import concourse.bacc as bacc
import concourse.bass as bass
import concourse.mybir as mybir
import concourse.tile as tile
from concourse import bass_utils
from concourse.masks import make_identity

N_CORES = 8
B, S, D, E, H = 4, 4096, 256, 8, 512
T = B * S                      # 16384 tokens total
TC = T // N_CORES              # 2048 tokens per core
CAP = 640                      # per-expert token capacity (mean 512, +6.5 sigma)
NCH = 16                       # 128-token chunks per core
DC = D // 128                  # 2
HC = H // 128                  # 4
TRASH = E * CAP                # overflow slot base
NXC = E * CAP + 16             # xc/yc row count (incl. trash rows)

F32 = mybir.dt.float32
F32R = mybir.dt.float32r
BF16 = mybir.dt.bfloat16
I16 = mybir.dt.int16
I32 = mybir.dt.int32
GELU = mybir.ActivationFunctionType.Gelu
IDENT = mybir.ActivationFunctionType.Identity
COPY = mybir.ActivationFunctionType.Copy
ALU = mybir.AluOpType


def _emit(tc: tile.TileContext, ctx: ExitStack, t_in: dict, scratch: dict, t_out):
    nc = tc.nc
    x_d = t_in["x"]
    wg_d, bg_d = t_in["Wg"], t_in["bg"]
    w1_d, b1_d = t_in["W1"], t_in["b1"]
    w2_d, b2_d = t_in["W2"], t_in["b2"]
    xc_d, yc_d = scratch["xc"], scratch["yc"]
    y_d = t_out

    singles = ctx.enter_context(tc.tile_pool(name="singles", bufs=1))
    xpool = ctx.enter_context(tc.tile_pool(name="xpool", bufs=1))
    hdpool = ctx.enter_context(tc.tile_pool(name="hdpool", bufs=1))
    xcpool = ctx.enter_context(tc.tile_pool(name="xcpool", bufs=2))
    hpool = ctx.enter_context(tc.tile_pool(name="hpool", bufs=2))
    ypool = ctx.enter_context(tc.tile_pool(name="ypool", bufs=4))
    opool = ctx.enter_context(tc.tile_pool(name="opool", bufs=1))
    ps_hd = ctx.enter_context(tc.tile_pool(name="ps_hd", bufs=2, space="PSUM"))
    ps_fa = ctx.enter_context(tc.tile_pool(name="ps_fa", bufs=2, space="PSUM"))
    ps_fb = ctx.enter_context(tc.tile_pool(name="ps_fb", bufs=2, space="PSUM"))
    ps_sm = ctx.enter_context(tc.tile_pool(name="ps_sm", bufs=2, space="PSUM"))

    def hd_ps():
        return ps_hd.tile([128, 512], F32, tag="hd", name="hd")

    # ---- constants ------------------------------------------------------
    ident = singles.tile([128, 128], F32)
    make_identity(nc, ident[:])
    ident_bf = singles.tile([128, 128], BF16)
    make_identity(nc, ident_bf[:])
    # strict upper-triangular as stored [k, p]: 1 iff k < p  (so that
    # matmul gives out[p, e] = sum_{k<p} rhs[k, e])
    triu = singles.tile([128, 128], F32)
    nc.gpsimd.memset(triu[:], 1.0)
    # affine iota value = p*1 + f*(-1); keep where iota < 0 (k < p reversed:
    # stored [k(part), p(free)]: value = k - p; keep (k - p) < 0 -> 1 else 0
    nc.gpsimd.affine_select(
        out=triu[:], in_=triu[:], compare_op=ALU.is_gt, fill=0.0,
        base=0, pattern=[[1, 128]], channel_multiplier=-1,
    )
    # 16->128 replication matrix: R16[k, p] = 1 iff p % 16 == k
    r16 = singles.tile([16, 128], F32)
    nc.vector.memset(r16[:], 0.0)
    for g in range(8):
        nc.vector.tensor_copy(r16[:, g * 16:(g + 1) * 16], ident[:16, :16])

    # ---- weights / biases ----------------------------------------------
    w1_sb = singles.tile([128, E, DC, H], BF16)
    w2_sb = singles.tile([128, E, HC, D], BF16)
    wg_sb = singles.tile([128, DC, E], F32)
    nc.sync.dma_start(out=wg_sb[:], in_=wg_d[:])
    b1_sb = singles.tile([128, E, HC], F32)
    nc.sync.dma_start(out=b1_sb[:], in_=b1_d[:])
    b2_sb = singles.tile([E, D], F32R)
    nc.sync.dma_start(out=b2_sb[:], in_=b2_d[:])
    bg_sb = singles.tile([E, 1], F32)
    nc.sync.dma_start(out=bg_sb[:], in_=bg_d[:, None])

    # ---- zero-fill xc + yc trash rows ----------------------------------
    zt = singles.tile([128, 10, D], BF16)
    nc.vector.memset(zt[:], 0.0)

    # ---- x loads + transposes + gate, pipelined per 512-token tile -----
    # x_t[p, g] = token g*128 + p   (for gate path)
    x_t = xpool.tile([128, NCH, D], F32, name="x_t")
    xv = x_d.rearrange("(g p) d -> p g d", p=128)
    for s in range(4):
        nc.sync.dma_start(out=x_t[:, s * 4:(s + 1) * 4, :],
                          in_=xv[:, s * 4:(s + 1) * 4, :])
    for i in range(4):                      # zero xc rows [0, 5120)
        nc.sync.dma_start(
            out=xc_d[i * 1280:(i + 1) * 1280, :].rearrange(
                "(p c) d -> p c d", c=10),
            in_=zt[:],
        )
    nc.sync.dma_start(out=xc_d[TRASH:NXC, :], in_=zt[:16, 0, :])
    nc.sync.dma_start(out=yc_d[TRASH:NXC, :], in_=zt[:16, 0, :])
    xT = xpool.tile([128, DC, TC], F32, name="xT")
    g_sb = hdpool.tile([E, TC], F32, name="g_sb")
    for s in range(4):
        for g in range(s * 4, (s + 1) * 4):
            for dc in range(DC):
                ps_t = hd_ps()
                nc.tensor.transpose(
                    out=ps_t[:, :128], in_=x_t[:, g, dc * 128:(dc + 1) * 128],
                    identity=ident[:],
                )
                nc.vector.tensor_copy(
                    xT[:, dc, g * 128:(g + 1) * 128], ps_t[:, :128])
        ps_gate = hd_ps()
        for dc in range(DC):
            nc.tensor.matmul(
                ps_gate[:E, :], wg_sb[:, dc, :], xT[:, dc, s * 512:(s + 1) * 512],
                start=(dc == 0), stop=(dc == DC - 1),
            )
        nc.scalar.activation(
            g_sb[:, s * 512:(s + 1) * 512], ps_gate[:E, :], IDENT, bias=bg_sb[:, 0:1]
        )

    # x_sc/weights issued after the gate-critical x_t load:
    # x_sc row i at (i%128, i//128) = token lam(i) = (i%16)*128 + i//16
    x_sc = xpool.tile([128, NCH, D], F32, name="x_sc")
    nc.scalar.dma_start(
        out=x_sc[:], in_=x_d.rearrange("(q g w) d -> w q g d", q=16, g=16, w=8)
    )
    x_bf = xpool.tile([128, NCH, D], BF16, name="x_bf")
    nc.scalar.activation(x_bf[:], x_sc[:], COPY)
    for e in range(E // 2):
        nc.scalar.dma_start(out=w1_sb[:, e], in_=w1_d[:, e])
        nc.scalar.dma_start(out=w2_sb[:, e], in_=w2_d[:, e])

    # ---- top-2 mask + slot machinery -----------------------------------
    # gtok[:, ch, :] = scores of token ch*128+p
    gtok = hdpool.tile([128, NCH, E], F32, name="gtok")
    for ch in range(NCH):
        ps_t = hd_ps()
        nc.tensor.transpose(
            out=ps_t[:, :E], in_=g_sb[:, ch * 128:(ch + 1) * 128],
            identity=ident[:E, :E],
        )
        nc.vector.tensor_copy(gtok[:, ch, :], ps_t[:, :E])
    m8 = hdpool.tile([128, NCH, 8], F32, name="m8")
    for ch in range(NCH):
        nc.vector.max(m8[:, ch, :], gtok[:, ch, :])
    mask_all = hdpool.tile([128, NCH, E], F32, name="mask_all")
    nc.vector.tensor_tensor(
        out=mask_all[:], in0=gtok[:], in1=m8[:, :, 1:2].to_broadcast([128, NCH, E]),
        op=ALU.is_ge,
    )
    sel0 = hdpool.tile([128, NCH, E], F32, name="sel0")
    nc.vector.tensor_tensor(
        out=sel0[:], in0=gtok[:], in1=m8[:, :, 0:1].to_broadcast([128, NCH, E]),
        op=ALU.is_ge,
    )
    sel1 = hdpool.tile([128, NCH, E], F32, name="sel1")
    nc.vector.tensor_tensor(out=sel1[:], in0=mask_all[:], in1=sel0[:], op=ALU.subtract)

    # inclusive cumsum over chunk axis (Hillis-Steele, ping-pong)
    cs = [hdpool.tile([128, NCH, E], F32, name=f"cs{i}") for i in range(2)]
    src = mask_all
    for i, d in enumerate((1, 2, 4, 8)):
        dst = cs[i % 2]
        nc.vector.tensor_tensor(
            out=dst[:, d:, :], in0=src[:, d:, :], in1=src[:, :NCH - d, :], op=ALU.add
        )
        nc.vector.tensor_copy(dst[:, :d, :], src[:, :d, :])
        src = dst
    csum = src                                   # [128, NCH, E] inclusive

    # partition-exclusive prefix of row totals
    rowtot = hdpool.tile([128, E], F32, name="rowtot")
    nc.vector.tensor_copy(rowtot[:], csum[:, NCH - 1, :])
    ps_pref = hd_ps()
    nc.tensor.matmul(ps_pref[:, :E], triu[:], rowtot[:], start=True, stop=True)

    # base[p, e] = pref[p, e] + e*CAP
    eoff = hdpool.tile([128, E], F32, name="eoff")
    for e in range(E):
        nc.vector.memset(eoff[:, e:e + 1], float(e * CAP))
    base_sb = hdpool.tile([128, E], F32, name="base")
    nc.vector.tensor_tensor(out=base_sb[:], in0=ps_pref[:, :E], in1=eoff[:], op=ALU.add)

    # oslot[t, e] = excl-cumsum + base ; overflow -> TRASH
    excl = hdpool.tile([128, NCH, E], F32, name="excl")
    nc.vector.tensor_tensor(out=excl[:], in0=csum[:], in1=mask_all[:], op=ALU.subtract)
    oslot = hdpool.tile([128, NCH, E], F32, name="oslot")
    nc.vector.tensor_tensor(
        out=oslot[:], in0=excl[:],
        in1=base_sb[:, None, :].to_broadcast([128, NCH, E]), op=ALU.add,
    )
    ov01 = hdpool.tile([128, NCH, E], F32, name="ov01")
    nc.vector.tensor_scalar(
        out=ov01[:], in0=excl[:], scalar1=float(CAP), scalar2=None, op0=ALU.is_ge
    )
    notover = hdpool.tile([128, NCH, E], F32, name="notover")
    nc.vector.tensor_scalar(
        out=notover[:], in0=ov01[:], scalar1=-1.0, scalar2=1.0,
        op0=ALU.mult, op1=ALU.add,
    )
    ovtrash = hdpool.tile([128, NCH, E], F32, name="ovtrash")
    nc.vector.tensor_scalar(
        out=ovtrash[:], in0=ov01[:], scalar1=float(TRASH), scalar2=None,
        op0=ALU.mult,
    )
    nc.vector.tensor_tensor(out=oslot[:], in0=oslot[:], in1=notover[:], op=ALU.mult)
    nc.vector.tensor_tensor(out=oslot[:], in0=oslot[:], in1=ovtrash[:], op=ALU.add)

    # per rank: slot list -> idx list -> scatter immediately (one 2048-desc
    # call per rank; descs proven safe up to 2048 for scatter)
    idx = []
    for r, sel in ((0, sel0), (1, sel1)):
        tmp = hdpool.tile([128, NCH, E], F32, name=f"otmp{r}")
        nc.vector.tensor_tensor(out=tmp[:], in0=sel[:], in1=oslot[:], op=ALU.mult)
        o_r = hdpool.tile([128, NCH], F32, name=f"o{r}")
        nc.vector.tensor_reduce(o_r[:], tmp[:], axis=mybir.AxisListType.X, op=ALU.add)
        ps_t = hd_ps()
        nc.tensor.transpose(out=ps_t[:16, :128], in_=o_r[:], identity=ident[:])
        t_sb = hdpool.tile([16, 128], F32, name=f"t_sb{r}")
        nc.vector.tensor_copy(t_sb[:], ps_t[:16, :128])
        ps_r = hd_ps()
        nc.tensor.matmul(ps_r[:, :128], r16[:], t_sb[:], start=True, stop=True)
        idx_r = hdpool.tile([128, 128], I16, name=f"idx{r}")
        nc.vector.tensor_copy(idx_r[:], ps_r[:, :128])
        idx.append(idx_r)
        nc.gpsimd.dma_scatter_add(
            out_ap=xc_d[:, :], in_ap=x_bf[:], idxs_ap=idx_r[:],
            num_idxs=TC, num_idxs_reg=TC, elem_size=D, queue_num=0,
        )

    # FM gather index lists: slot lists e*CAP + [0..CAP) in 16-wrap layout
    base16 = hdpool.tile([16, CAP // 16], F32, name="base16")
    for f in range(CAP // 16):
        nc.vector.memset(base16[:, f:f + 1], float(f * 16))
    # qcol[q] = q: reduce(ident16 * [16f row values]) / 16
    qtmp = hdpool.tile([16, 16], F32, name="qtmp")
    nc.vector.tensor_tensor(
        out=qtmp[:], in0=ident[:16, :16], in1=base16[:, :16], op=ALU.mult
    )
    qcol = hdpool.tile([16, 1], F32, name="qcol")
    nc.vector.tensor_reduce(qcol[:], qtmp[:], axis=mybir.AxisListType.X, op=ALU.add)
    nc.vector.tensor_scalar(
        out=qcol[:], in0=qcol[:], scalar1=1.0 / 16.0, scalar2=None, op0=ALU.mult
    )
    nc.vector.tensor_tensor(
        out=base16[:], in0=base16[:],
        in1=qcol[:].to_broadcast([16, CAP // 16]), op=ALU.add,
    )
    ps_bi = hd_ps()
    nc.tensor.matmul(ps_bi[:, :CAP // 16], r16[:], base16[:], start=True, stop=True)
    base128 = hdpool.tile([128, CAP // 16], F32, name="base128")
    nc.vector.tensor_copy(base128[:], ps_bi[:, :CAP // 16])
    idx_fm = []
    for e in range(E):
        fme = hdpool.tile([128, CAP // 16], F32, name=f"fme{e}")
        nc.vector.tensor_scalar(
            out=fme[:], in0=base128[:], scalar1=float(e * CAP), scalar2=None,
            op0=ALU.add,
        )
        ie = hdpool.tile([128, CAP // 16], I16, name=f"ifm{e}")
        nc.vector.tensor_copy(ie[:], fme[:])
        idx_fm.append(ie)

    # mt3[e, f, q] = mask[token q*128+f, e]; b2tok = mask x b2 token-major,
    # precomputed here (PE has slack) to keep the combine tail lean.
    mt3 = hdpool.tile([E, 128, NCH], F32R, name="mt3")
    for ch in range(NCH):
        ps_mt = hd_ps()
        nc.tensor.transpose(
            out=ps_mt[:E, :128], in_=mask_all[:, ch, :], identity=ident[:]
        )
        nc.vector.tensor_copy(mt3[:, :, ch], ps_mt[:E, :128])
    b2tok = hdpool.tile([128, NCH, D], F32, name="b2tok")
    for fb in range(NCH):
        ps_b2 = hd_ps()
        nc.tensor.matmul(
            ps_b2[:, :D], mt3[:, fb * 8:(fb + 1) * 8, :], b2_sb[:],
            start=True, stop=True,
        )
        nc.scalar.activation(b2tok[:, fb, :], ps_b2[:, :D], COPY)

    def _debug_out():
        outsb = opool.tile([128, NCH, D], F32, name="outsb_dbg")
        nc.vector.memset(outsb[:], 0.0)
        nc.sync.dma_start(
            out=y_d.rearrange("(q g w) d -> w q g d", q=16, g=16, w=8),
            in_=outsb[:],
        )

    if STOP == "head":
        _debug_out()
        return

    if "dbg_idx" in scratch:
        for r in range(2):
            nc.sync.dma_start(
                out=scratch["dbg_idx"][r * 128:(r + 1) * 128, :], in_=idx[r][:])

    if STOP == "scatter":
        _debug_out()
        return

    # ---- per-expert FFN -------------------------------------------------
    for e in range(E):
        if e == 0:
            for e2 in range(E // 2, E):
                nc.sync.dma_start(out=w1_sb[:, e2], in_=w1_d[:, e2])
                nc.sync.dma_start(out=w2_sb[:, e2], in_=w2_d[:, e2])
        xcT = xcpool.tile([128, DC, CAP], BF16, tag="xcT", name=f"xcT{e}")
        nc.gpsimd.dma_gather(
            out_ap=xcT[:], in_ap=xc_d[:, :], idxs_ap=idx_fm[e][:],
            num_idxs=CAP, num_idxs_reg=CAP, elem_size=D, transpose=True,
            queue_num=0,
        )

        h_sb = hpool.tile([128, HC, CAP], BF16, tag="h", name=f"h{e}")
        for hc in range(HC):
            ps_ha = ps_fa.tile([128, 512], F32, tag="fmA")
            ps_hb = ps_fb.tile([128, 128], F32, tag="fmB")
            for dc in range(DC):
                nc.tensor.matmul(
                    ps_ha[:], w1_sb[:, e, dc, hc * 128:(hc + 1) * 128],
                    xcT[:, dc, 0:512], start=(dc == 0), stop=(dc == DC - 1),
                )
            for dc in range(DC):
                nc.tensor.matmul(
                    ps_hb[:], w1_sb[:, e, dc, hc * 128:(hc + 1) * 128],
                    xcT[:, dc, 512:CAP], start=(dc == 0), stop=(dc == DC - 1),
                )
            nc.scalar.activation(
                h_sb[:, hc, 0:512], ps_ha[:], GELU, bias=b1_sb[:, e, hc:hc + 1]
            )
            nc.scalar.activation(
                h_sb[:, hc, 512:CAP], ps_hb[:], GELU, bias=b1_sb[:, e, hc:hc + 1]
            )
        # flipped second matmul: out[token(slot), d]
        for tb in range(CAP // 128):
            ps_o = ps_sm.tile([128, D], F32, tag="sm")
            for hc in range(HC):
                nc.tensor.matmul(
                    ps_o[:], h_sb[:, hc, tb * 128:(tb + 1) * 128],
                    w2_sb[:, e, hc, :], start=(hc == 0), stop=(hc == HC - 1),
                )
            yst = ypool.tile([128, D], BF16, tag="yst", name=f"yst{e}_{tb}")
            nc.scalar.activation(yst[:], ps_o[:], COPY)
            eng = nc.sync if (e * 5 + tb) % 2 == 0 else nc.scalar
            eng.dma_start(
                out=yc_d[e * CAP + tb * 128:e * CAP + (tb + 1) * 128, :],
                in_=yst[:],
            )

    if STOP == "ffn":
        _debug_out()
        return

    # ---- combine --------------------------------------------------------
    yT = [[None] * 4, [None] * 4]
    for c in range(4):
        for r in range(2):
            y_rc = opool.tile([128, DC, 512], BF16, tag=f"yT{r}_{c}",
                              name=f"yT{r}_{c}")
            nc.gpsimd.dma_gather(
                out_ap=y_rc[:], in_ap=yc_d[:, :],
                idxs_ap=idx[r][:, c * 32:(c + 1) * 32],
                num_idxs=512, num_idxs_reg=512, elem_size=D, transpose=True,
                queue_num=0,
            )
            yT[r][c] = y_rc
    if STOP == "gather":
        _debug_out()
        return

    ysum = opool.tile([128, DC, TC], F32, name="ysum")
    outsb = opool.tile([128, NCH, D], F32, name="outsb")
    for c in range(4):
        nc.vector.tensor_tensor(
            out=ysum[:, :, c * 512:(c + 1) * 512], in0=yT[0][c][:],
            in1=yT[1][c][:], op=ALU.add,
        )
        for fb in range(c * 4, (c + 1) * 4):
            ps_o = ps_sm.tile([128, D], F32, tag="sm")
            for dc in range(DC):
                nc.tensor.matmul(
                    ps_o[:, dc * 128:(dc + 1) * 128],
                    ysum[:, dc, fb * 128:(fb + 1) * 128], ident[:],
                    is_transpose=True, start=(dc == 0), stop=(dc == DC - 1),
                    skip_group_check=True,
                )
            nc.vector.tensor_tensor(
                out=outsb[:, fb, :], in0=ps_o[:], in1=b2tok[:, fb, :], op=ALU.add
            )
    nc.sync.dma_start(
        out=y_d.rearrange("(q g w) d -> w q g d", q=16, g=16, w=8),
        in_=outsb[:],
    )


_CACHE = {}


def _build():
    if "nc" in _CACHE:
        return _CACHE["nc"]
    nc = bacc.Bacc("TRN2", target_bir_lowering=False)
    t_in = {
        "x": nc.dram_tensor("x", [TC, D], F32, kind="ExternalInput"),
        "Wg": nc.dram_tensor("Wg", [128, DC, E], F32, kind="ExternalInput"),
        "bg": nc.dram_tensor("bg", [E], F32, kind="ExternalInput"),
        "W1": nc.dram_tensor("W1", [128, E, DC, H], BF16, kind="ExternalInput"),
        "b1": nc.dram_tensor("b1", [128, E, HC], F32, kind="ExternalInput"),
        "W2": nc.dram_tensor("W2", [128, E, HC, D], BF16, kind="ExternalInput"),
        "b2": nc.dram_tensor("b2", [E, D], F32R, kind="ExternalInput"),
    }
    dbg = os.environ.get("K_DEBUG") == "1"
    # NOTE: scratch must be ExternalOutput, not Internal: Internal DRAM
    # compiles to a fixed NEFF address, and the 8 SPMD cores share HBM --
    # every core would scatter into the same physical buffer.  External
    # outputs get per-core runtime allocations.
    scratch = {
        "xc": nc.dram_tensor("xc", [NXC, D], BF16, kind="ExternalOutput"),
        "yc": nc.dram_tensor("yc", [NXC, D], BF16, kind="ExternalOutput"),
    }
    if dbg:
        scratch["dbg_idx"] = nc.dram_tensor(
            "dbg_idx", [256, 128], mybir.dt.int16, kind="ExternalOutput")
    y_d = nc.dram_tensor("y", [TC, D], F32, kind="ExternalOutput")
    with tile.TileContext(nc) as tc:
        with ExitStack() as ctx:
            _emit(tc, ctx, t_in, scratch, y_d)
    nc.compile()
    _CACHE["nc"] = nc
    return nc


def _prep_shared(inputs):
    f = lambda a: np.ascontiguousarray(np.asarray(a, dtype=np.float32))
    bf = lambda a: np.ascontiguousarray(a.astype(ml_dtypes.bfloat16))
    w1 = f(inputs["W1"])
    w2 = f(inputs["W2"])
    wg = f(inputs["Wg"])
    b1 = f(inputs["b1"])
    return {
        "Wg": np.ascontiguousarray(wg.reshape(DC, 128, E).transpose(1, 0, 2)),
        "bg": f(inputs["bg"]),
        "W1": bf(w1.reshape(E, DC, 128, H).transpose(2, 0, 1, 3)),
        "b1": np.ascontiguousarray(b1.reshape(E, HC, 128).transpose(2, 0, 1)),
        "W2": bf(w2.reshape(E, HC, 128, D).transpose(2, 0, 1, 3)),
        "b2": f(inputs["b2"]),
    }


def _run(inputs: dict, trace: bool = False, **kw):
    nc = _build()
    x = np.ascontiguousarray(np.asarray(inputs["x"], dtype=np.float32)).reshape(T, D)
    shared = _prep_shared(inputs)
    in_maps = [
        {"x": x[c * TC:(c + 1) * TC], **shared} for c in range(N_CORES)
    ]
    br = bass_utils.run_bass_kernel_spmd(
        nc, in_maps, core_ids=list(range(N_CORES)), trace=trace, **kw
    )
    out = np.concatenate([r["y"] for r in br.results], axis=0)
    return out.reshape(B, S, D), br


def kernel(**inputs) -> np.ndarray:
    out, _ = _run(inputs, trace=False)
    return out


# revision 16
# speedup vs baseline: 1.1479x; 1.1251x over previous
"""MoE layer (top-2 of 8 experts) — routed Trainium2 Bass kernel.

Data-parallel over tokens across 8 NeuronCores (2048 tokens/core), expert
weights replicated (bf16).  Unlike the dense baseline (all 8 experts for
every token), this kernel routes: tokens are compacted into per-expert
DRAM buffers via dma_scatter_add, each expert's FFN runs only on its own
~512 tokens (capacity 640), and the top-2 combine is a pair of
dma_gather(transpose=True) reads + one DVE add.  4x fewer matmul rows.

Per-core pipeline:
  gate   : psum_g[8,512] = Wg.T @ X.T  (exact fp32) -> top-2 mask per token
  index  : chunk-wise cumsum (Hillis-Steele on DVE) + strict-triangular
           PE matmul -> per-(token,expert) slot; slot lists for the
           scatter/gather sides built with PE transposes + a 16->128
           replication matmul.
  scatter: dma_scatter_add of bf16 x rows into zeroed xc[e*CAP + slot]
           (2 calls: rank-0 / rank-1 slots; collision-free by construction)
  ffn    : per expert: dma_start_transpose loads xc slab feature-major;
           FM (bf16) + GELU(+b1); second matmul is flipped (h-block as
           lhsT) so the output is token(slot)-major -> plain DMA to yc.
  combine: dma_gather(transpose=True) y0T/y1T feature-major by slot list,
           DVE add, then per 128-token block: mask x b2 matmul into PSUM
           followed by accumulating PE transposes; activation copy out.
"""

import os
import sys

sys.path.insert(0, "/opt/trn_rl_repo")

STOP = os.environ.get("K_STOP", "full")   # head|scatter|ffn|gather|full

from contextlib import ExitStack

import numpy as np
import ml_dtypes

import concourse.bacc as bacc
import concourse.bass as bass
import concourse.mybir as mybir
import concourse.tile as tile
from concourse import bass_utils
from concourse.masks import make_identity

N_CORES = 8
B, S, D, E, H = 4, 4096, 256, 8, 512
T = B * S                      # 16384 tokens total
TC = T // N_CORES              # 2048 tokens per core
CAP = 640                      # per-expert token capacity (mean 512, +6.5 sigma)
NCH = 16                       # 128-token chunks per core
DC = D // 128                  # 2
HC = H // 128                  # 4
TRASH = E * CAP                # overflow slot base
NXC = E * CAP + 16             # xc/yc row count (incl. trash rows)

F32 = mybir.dt.float32
F32R = mybir.dt.float32r
BF16 = mybir.dt.bfloat16
I16 = mybir.dt.int16
I32 = mybir.dt.int32
GELU = mybir.ActivationFunctionType.Gelu
IDENT = mybir.ActivationFunctionType.Identity
COPY = mybir.ActivationFunctionType.Copy
ALU = mybir.AluOpType


def _emit(tc: tile.TileContext, ctx: ExitStack, t_in: dict, scratch: dict, t_out):
    nc = tc.nc
    x_d = t_in["x"]
    wg_d, bg_d = t_in["Wg"], t_in["bg"]
    w1_d, b1_d = t_in["W1"], t_in["b1"]
    w2_d, b2_d = t_in["W2"], t_in["b2"]
    xc_d, yc_d = scratch["xc"], scratch["yc"]
    y_d = t_out

    singles = ctx.enter_context(tc.tile_pool(name="singles", bufs=1))
    xpool = ctx.enter_context(tc.tile_pool(name="xpool", bufs=1))
    hdpool = ctx.enter_context(tc.tile_pool(name="hdpool", bufs=1))
    xcpool = ctx.enter_context(tc.tile_pool(name="xcpool", bufs=2))
    hpool = ctx.enter_context(tc.tile_pool(name="hpool", bufs=2))
    ypool = ctx.enter_context(tc.tile_pool(name="ypool", bufs=4))
    opool = ctx.enter_context(tc.tile_pool(name="opool", bufs=1))
    ps_hd = ctx.enter_context(tc.tile_pool(name="ps_hd", bufs=2, space="PSUM"))
    ps_fa = ctx.enter_context(tc.tile_pool(name="ps_fa", bufs=2, space="PSUM"))
    ps_fb = ctx.enter_context(tc.tile_pool(name="ps_fb", bufs=2, space="PSUM"))
    ps_sm = ctx.enter_context(tc.tile_pool(name="ps_sm", bufs=2, space="PSUM"))

    def hd_ps():
        return ps_hd.tile([128, 512], F32, tag="hd", name="hd")

    # ---- constants ------------------------------------------------------
    ident = singles.tile([128, 128], F32)
    make_identity(nc, ident[:])
    ident_bf = singles.tile([128, 128], BF16)
    make_identity(nc, ident_bf[:])
    # strict upper-triangular as stored [k, p]: 1 iff k < p  (so that
    # matmul gives out[p, e] = sum_{k<p} rhs[k, e])
    triu = singles.tile([128, 128], F32)
    nc.gpsimd.memset(triu[:], 1.0)
    # affine iota value = p*1 + f*(-1); keep where iota < 0 (k < p reversed:
    # stored [k(part), p(free)]: value = k - p; keep (k - p) < 0 -> 1 else 0
    nc.gpsimd.affine_select(
        out=triu[:], in_=triu[:], compare_op=ALU.is_gt, fill=0.0,
        base=0, pattern=[[1, 128]], channel_multiplier=-1,
    )
    # 16->128 replication matrix: R16[k, p] = 1 iff p % 16 == k
    r16 = singles.tile([16, 128], F32)
    nc.vector.memset(r16[:], 0.0)
    for g in range(8):
        nc.vector.tensor_copy(r16[:, g * 16:(g + 1) * 16], ident[:16, :16])

    # ---- weights / biases ----------------------------------------------
    w1_sb = singles.tile([128, E, DC, H], BF16)
    w2_sb = singles.tile([128, E, HC, D], BF16)
    wg_sb = singles.tile([128, DC, E], F32)
    nc.sync.dma_start(out=wg_sb[:], in_=wg_d[:])
    b1_sb = singles.tile([128, E, HC], F32)
    nc.sync.dma_start(out=b1_sb[:], in_=b1_d[:])
    b2_sb = singles.tile([E, D], F32R)
    nc.sync.dma_start(out=b2_sb[:], in_=b2_d[:])
    bg_sb = singles.tile([E, 1], F32)
    nc.sync.dma_start(out=bg_sb[:], in_=bg_d[:, None])

    # ---- zero-fill xc + yc trash rows ----------------------------------
    zt = singles.tile([128, 10, D], BF16)
    nc.vector.memset(zt[:], 0.0)

    # ---- x loads + transposes + gate, pipelined per 512-token tile -----
    # x_t[p, g] = token g*128 + p   (for gate path)
    x_t = xpool.tile([128, NCH, D], F32, name="x_t")
    xv = x_d.rearrange("(p g) d -> p g d", p=128)
    for s in range(4):
        nc.sync.dma_start(out=x_t[:, s * 4:(s + 1) * 4, :],
                          in_=xv[:, s * 4:(s + 1) * 4, :])
    for i in range(4):                      # zero xc rows [0, 5120)
        nc.sync.dma_start(
            out=xc_d[i * 1280:(i + 1) * 1280, :].rearrange(
                "(p c) d -> p c d", c=10),
            in_=zt[:],
        )
    nc.sync.dma_start(out=xc_d[TRASH:NXC, :], in_=zt[:16, 0, :])
    nc.sync.dma_start(out=yc_d[TRASH:NXC, :], in_=zt[:16, 0, :])
    xT = xpool.tile([128, DC, TC], F32, name="xT")
    g_sb = hdpool.tile([E, TC], F32, name="g_sb")
    for s in range(4):
        for g in range(s * 4, (s + 1) * 4):
            for dc in range(DC):
                ps_t = hd_ps()
                nc.tensor.transpose(
                    out=ps_t[:, :128], in_=x_t[:, g, dc * 128:(dc + 1) * 128],
                    identity=ident[:],
                )
                nc.vector.tensor_copy(
                    xT[:, dc, g * 128:(g + 1) * 128], ps_t[:, :128])
        ps_gate = hd_ps()
        for dc in range(DC):
            nc.tensor.matmul(
                ps_gate[:E, :], wg_sb[:, dc, :], xT[:, dc, s * 512:(s + 1) * 512],
                start=(dc == 0), stop=(dc == DC - 1),
            )
        nc.scalar.activation(
            g_sb[:, s * 512:(s + 1) * 512], ps_gate[:E, :], IDENT, bias=bg_sb[:, 0:1]
        )

    # x_sc/weights issued after the gate-critical x_t load.
    # With token tau = p*16 + ch (x_t contiguous), list position i maps to
    # token i, so the scatter source needs x_sc[p, g] = x[g*128 + p].
    x_sc = xpool.tile([128, NCH, D], F32, name="x_sc")
    nc.scalar.dma_start(
        out=x_sc[:], in_=x_d.rearrange("(g p) d -> p g d", p=128)
    )
    x_bf = xpool.tile([128, NCH, D], BF16, name="x_bf")
    nc.scalar.activation(x_bf[:], x_sc[:], COPY)
    for e in range(E // 2):
        nc.scalar.dma_start(out=w1_sb[:, e], in_=w1_d[:, e])
        nc.scalar.dma_start(out=w2_sb[:, e], in_=w2_d[:, e])

    # ---- top-2 mask + slot machinery -----------------------------------
    # gtok[:, ch, :] = scores of token ch*128+p
    gtok = hdpool.tile([128, NCH, E], F32, name="gtok")
    for ch in range(NCH):
        ps_t = hd_ps()
        nc.tensor.transpose(
            out=ps_t[:, :E], in_=g_sb[:, ch * 128:(ch + 1) * 128],
            identity=ident[:E, :E],
        )
        nc.vector.tensor_copy(gtok[:, ch, :], ps_t[:, :E])
    m8 = hdpool.tile([128, NCH, 8], F32, name="m8")
    for ch in range(NCH):
        nc.vector.max(m8[:, ch, :], gtok[:, ch, :])
    mask_all = hdpool.tile([128, NCH, E], F32, name="mask_all")
    nc.vector.tensor_tensor(
        out=mask_all[:], in0=gtok[:], in1=m8[:, :, 1:2].to_broadcast([128, NCH, E]),
        op=ALU.is_ge,
    )
    sel0 = hdpool.tile([128, NCH, E], F32, name="sel0")
    nc.vector.tensor_tensor(
        out=sel0[:], in0=gtok[:], in1=m8[:, :, 0:1].to_broadcast([128, NCH, E]),
        op=ALU.is_ge,
    )
    sel1 = hdpool.tile([128, NCH, E], F32, name="sel1")
    nc.vector.tensor_tensor(out=sel1[:], in0=mask_all[:], in1=sel0[:], op=ALU.subtract)

    # inclusive cumsum over chunk axis (Hillis-Steele, ping-pong)
    cs = [hdpool.tile([128, NCH, E], F32, name=f"cs{i}") for i in range(2)]
    src = mask_all
    for i, d in enumerate((1, 2, 4, 8)):
        dst = cs[i % 2]
        nc.vector.tensor_tensor(
            out=dst[:, d:, :], in0=src[:, d:, :], in1=src[:, :NCH - d, :], op=ALU.add
        )
        nc.vector.tensor_copy(dst[:, :d, :], src[:, :d, :])
        src = dst
    csum = src                                   # [128, NCH, E] inclusive

    # partition-exclusive prefix of row totals
    rowtot = hdpool.tile([128, E], F32, name="rowtot")
    nc.vector.tensor_copy(rowtot[:], csum[:, NCH - 1, :])
    ps_pref = hd_ps()
    nc.tensor.matmul(ps_pref[:, :E], triu[:], rowtot[:], start=True, stop=True)

    # base[p, e] = pref[p, e] + e*CAP
    eoff = hdpool.tile([128, E], F32, name="eoff")
    for e in range(E):
        nc.vector.memset(eoff[:, e:e + 1], float(e * CAP))
    base_sb = hdpool.tile([128, E], F32, name="base")
    nc.vector.tensor_tensor(out=base_sb[:], in0=ps_pref[:, :E], in1=eoff[:], op=ALU.add)

    # oslot[t, e] = excl-cumsum + base ; overflow -> TRASH
    excl = hdpool.tile([128, NCH, E], F32, name="excl")
    nc.vector.tensor_tensor(out=excl[:], in0=csum[:], in1=mask_all[:], op=ALU.subtract)
    oslot = hdpool.tile([128, NCH, E], F32, name="oslot")
    nc.vector.tensor_tensor(
        out=oslot[:], in0=excl[:],
        in1=base_sb[:, None, :].to_broadcast([128, NCH, E]), op=ALU.add,
    )
    ov01 = hdpool.tile([128, NCH, E], F32, name="ov01")
    nc.vector.tensor_scalar(
        out=ov01[:], in0=excl[:], scalar1=float(CAP), scalar2=None, op0=ALU.is_ge
    )
    notover = hdpool.tile([128, NCH, E], F32, name="notover")
    nc.vector.tensor_scalar(
        out=notover[:], in0=ov01[:], scalar1=-1.0, scalar2=1.0,
        op0=ALU.mult, op1=ALU.add,
    )
    ovtrash = hdpool.tile([128, NCH, E], F32, name="ovtrash")
    nc.vector.tensor_scalar(
        out=ovtrash[:], in0=ov01[:], scalar1=float(TRASH), scalar2=None,
        op0=ALU.mult,
    )
    nc.vector.tensor_tensor(out=oslot[:], in0=oslot[:], in1=notover[:], op=ALU.mult)
    nc.vector.tensor_tensor(out=oslot[:], in0=oslot[:], in1=ovtrash[:], op=ALU.add)

    # per rank: slot list -> idx list -> scatter immediately (one 2048-desc
    # call per rank; descs proven safe up to 2048 for scatter)
    idx = []
    for r, sel in ((0, sel0), (1, sel1)):
        tmp = hdpool.tile([128, NCH, E], F32, name=f"otmp{r}")
        nc.vector.tensor_tensor(out=tmp[:], in0=sel[:], in1=oslot[:], op=ALU.mult)
        o_r = hdpool.tile([128, NCH], F32, name=f"o{r}")
        nc.vector.tensor_reduce(o_r[:], tmp[:], axis=mybir.AxisListType.X, op=ALU.add)
        ps_t = hd_ps()
        nc.tensor.transpose(out=ps_t[:16, :128], in_=o_r[:], identity=ident[:])
        t_sb = hdpool.tile([16, 128], F32, name=f"t_sb{r}")
        nc.vector.tensor_copy(t_sb[:], ps_t[:16, :128])
        ps_r = hd_ps()
        nc.tensor.matmul(ps_r[:, :128], r16[:], t_sb[:], start=True, stop=True)
        idx_r = hdpool.tile([128, 128], I16, name=f"idx{r}")
        nc.vector.tensor_copy(idx_r[:], ps_r[:, :128])
        idx.append(idx_r)
        nc.gpsimd.dma_scatter_add(
            out_ap=xc_d[:, :], in_ap=x_bf[:], idxs_ap=idx_r[:],
            num_idxs=TC, num_idxs_reg=TC, elem_size=D, queue_num=0,
        )

    # FM gather index lists: slot lists e*CAP + [0..CAP) in 16-wrap layout
    base16 = hdpool.tile([16, CAP // 16], F32, name="base16")
    for f in range(CAP // 16):
        nc.vector.memset(base16[:, f:f + 1], float(f * 16))
    # qcol[q] = q: reduce(ident16 * [16f row values]) / 16
    qtmp = hdpool.tile([16, 16], F32, name="qtmp")
    nc.vector.tensor_tensor(
        out=qtmp[:], in0=ident[:16, :16], in1=base16[:, :16], op=ALU.mult
    )
    qcol = hdpool.tile([16, 1], F32, name="qcol")
    nc.vector.tensor_reduce(qcol[:], qtmp[:], axis=mybir.AxisListType.X, op=ALU.add)
    nc.vector.tensor_scalar(
        out=qcol[:], in0=qcol[:], scalar1=1.0 / 16.0, scalar2=None, op0=ALU.mult
    )
    nc.vector.tensor_tensor(
        out=base16[:], in0=base16[:],
        in1=qcol[:].to_broadcast([16, CAP // 16]), op=ALU.add,
    )
    ps_bi = hd_ps()
    nc.tensor.matmul(ps_bi[:, :CAP // 16], r16[:], base16[:], start=True, stop=True)
    base128 = hdpool.tile([128, CAP // 16], F32, name="base128")
    nc.vector.tensor_copy(base128[:], ps_bi[:, :CAP // 16])
    idx_fm = []
    for e in range(E):
        fme = hdpool.tile([128, CAP // 16], F32, name=f"fme{e}")
        nc.vector.tensor_scalar(
            out=fme[:], in0=base128[:], scalar1=float(e * CAP), scalar2=None,
            op0=ALU.add,
        )
        ie = hdpool.tile([128, CAP // 16], I16, name=f"ifm{e}")
        nc.vector.tensor_copy(ie[:], fme[:])
        idx_fm.append(ie)

    # mt3[e, f, q] = mask[token q*128+f, e]; b2tok = mask x b2 token-major,
    # precomputed here (PE has slack) to keep the combine tail lean.
    mt3 = hdpool.tile([E, 128, NCH], F32R, name="mt3")
    for ch in range(NCH):
        ps_mt = hd_ps()
        nc.tensor.transpose(
            out=ps_mt[:E, :128], in_=mask_all[:, ch, :], identity=ident[:]
        )
        nc.vector.tensor_copy(mt3[:, :, ch], ps_mt[:E, :128])
    b2tok = hdpool.tile([128, NCH, D], F32, name="b2tok")
    for fb in range(NCH):
        ps_b2 = hd_ps()
        nc.tensor.matmul(
            ps_b2[:, :D], mt3[:, fb * 8:(fb + 1) * 8, :], b2_sb[:],
            start=True, stop=True,
        )
        nc.scalar.activation(b2tok[:, fb, :], ps_b2[:, :D], COPY)

    def _debug_out():
        outsb = opool.tile([128, NCH, D], F32, name="outsb_dbg")
        nc.vector.memset(outsb[:], 0.0)
        nc.sync.dma_start(
            out=y_d.rearrange("(g p) d -> p g d", p=128),
            in_=outsb[:],
        )

    if STOP == "head":
        _debug_out()
        return

    if "dbg_idx" in scratch:
        for r in range(2):
            nc.sync.dma_start(
                out=scratch["dbg_idx"][r * 128:(r + 1) * 128, :], in_=idx[r][:])

    if STOP == "scatter":
        _debug_out()
        return

    # ---- per-expert FFN -------------------------------------------------
    for e in range(E):
        if e == 0:
            for e2 in range(E // 2, E):
                nc.sync.dma_start(out=w1_sb[:, e2], in_=w1_d[:, e2])
                nc.sync.dma_start(out=w2_sb[:, e2], in_=w2_d[:, e2])
        xcT = xcpool.tile([128, DC, CAP], BF16, tag="xcT", name=f"xcT{e}")
        nc.gpsimd.dma_gather(
            out_ap=xcT[:], in_ap=xc_d[:, :], idxs_ap=idx_fm[e][:],
            num_idxs=CAP, num_idxs_reg=CAP, elem_size=D, transpose=True,
            queue_num=0,
        )

        h_sb = hpool.tile([128, HC, CAP], BF16, tag="h", name=f"h{e}")
        for hc in range(HC):
            ps_ha = ps_fa.tile([128, 512], F32, tag="fmA")
            ps_hb = ps_fb.tile([128, 128], F32, tag="fmB")
            for dc in range(DC):
                nc.tensor.matmul(
                    ps_ha[:], w1_sb[:, e, dc, hc * 128:(hc + 1) * 128],
                    xcT[:, dc, 0:512], start=(dc == 0), stop=(dc == DC - 1),
                )
            for dc in range(DC):
                nc.tensor.matmul(
                    ps_hb[:], w1_sb[:, e, dc, hc * 128:(hc + 1) * 128],
                    xcT[:, dc, 512:CAP], start=(dc == 0), stop=(dc == DC - 1),
                )
            nc.scalar.activation(
                h_sb[:, hc, 0:512], ps_ha[:], GELU, bias=b1_sb[:, e, hc:hc + 1]
            )
            nc.scalar.activation(
                h_sb[:, hc, 512:CAP], ps_hb[:], GELU, bias=b1_sb[:, e, hc:hc + 1]
            )
        # flipped second matmul: out[token(slot), d]
        for tb in range(CAP // 128):
            ps_o = ps_sm.tile([128, D], F32, tag="sm")
            for hc in range(HC):
                nc.tensor.matmul(
                    ps_o[:], h_sb[:, hc, tb * 128:(tb + 1) * 128],
                    w2_sb[:, e, hc, :], start=(hc == 0), stop=(hc == HC - 1),
                )
            yst = ypool.tile([128, D], BF16, tag="yst", name=f"yst{e}_{tb}")
            nc.scalar.activation(yst[:], ps_o[:], COPY)
            eng = nc.sync if (e * 5 + tb) % 2 == 0 else nc.scalar
            eng.dma_start(
                out=yc_d[e * CAP + tb * 128:e * CAP + (tb + 1) * 128, :],
                in_=yst[:],
            )

    if STOP == "ffn":
        _debug_out()
        return

    # ---- combine --------------------------------------------------------
    yT = [[None] * 4, [None] * 4]
    for c in range(4):
        for r in range(2):
            y_rc = opool.tile([128, DC, 512], BF16, tag=f"yT{r}_{c}",
                              name=f"yT{r}_{c}")
            nc.gpsimd.dma_gather(
                out_ap=y_rc[:], in_ap=yc_d[:, :],
                idxs_ap=idx[r][:, c * 32:(c + 1) * 32],
                num_idxs=512, num_idxs_reg=512, elem_size=D, transpose=True,
                queue_num=0,
            )
            yT[r][c] = y_rc
    if STOP == "gather":
        _debug_out()
        return

    ysum = opool.tile([128, DC, TC], F32, name="ysum")
    outsb = opool.tile([128, NCH, D], F32, name="outsb")
    for c in range(4):
        nc.vector.tensor_tensor(
            out=ysum[:, :, c * 512:(c + 1) * 512], in0=yT[0][c][:],
            in1=yT[1][c][:], op=ALU.add,
        )
        for fb in range(c * 4, (c + 1) * 4):
            ps_o = ps_sm.tile([128, D], F32, tag="sm")
            for dc in range(DC):
                nc.tensor.matmul(
                    ps_o[:, dc * 128:(dc + 1) * 128],
                    ysum[:, dc, fb * 128:(fb + 1) * 128], ident[:],
                    is_transpose=True, start=(dc == 0), stop=(dc == DC - 1),
                    skip_group_check=True,
                )
            nc.vector.tensor_tensor(
                out=outsb[:, fb, :], in0=ps_o[:], in1=b2tok[:, fb, :], op=ALU.add
            )
    nc.sync.dma_start(
        out=y_d.rearrange("(g p) d -> p g d", p=128),
        in_=outsb[:],
    )


_CACHE = {}


def _build():
    if "nc" in _CACHE:
        return _CACHE["nc"]
    nc = bacc.Bacc("TRN2", target_bir_lowering=False)
    t_in = {
        "x": nc.dram_tensor("x", [TC, D], F32, kind="ExternalInput"),
        "Wg": nc.dram_tensor("Wg", [128, DC, E], F32, kind="ExternalInput"),
        "bg": nc.dram_tensor("bg", [E], F32, kind="ExternalInput"),
        "W1": nc.dram_tensor("W1", [128, E, DC, H], BF16, kind="ExternalInput"),
        "b1": nc.dram_tensor("b1", [128, E, HC], F32, kind="ExternalInput"),
        "W2": nc.dram_tensor("W2", [128, E, HC, D], BF16, kind="ExternalInput"),
        "b2": nc.dram_tensor("b2", [E, D], F32R, kind="ExternalInput"),
    }
    dbg = os.environ.get("K_DEBUG") == "1"
    # NOTE: scratch must be ExternalOutput, not Internal: Internal DRAM
    # compiles to a fixed NEFF address, and the 8 SPMD cores share HBM --
    # every core would scatter into the same physical buffer.  External
    # outputs get per-core runtime allocations.
    scratch = {
        "xc": nc.dram_tensor("xc", [NXC, D], BF16, kind="ExternalOutput"),
        "yc": nc.dram_tensor("yc", [NXC, D], BF16, kind="ExternalOutput"),
    }
    if dbg:
        scratch["dbg_idx"] = nc.dram_tensor(
            "dbg_idx", [256, 128], mybir.dt.int16, kind="ExternalOutput")
    y_d = nc.dram_tensor("y", [TC, D], F32, kind="ExternalOutput")
    with tile.TileContext(nc) as tc:
        with ExitStack() as ctx:
            _emit(tc, ctx, t_in, scratch, y_d)
    nc.compile()
    _CACHE["nc"] = nc
    return nc


def _prep_shared(inputs):
    f = lambda a: np.ascontiguousarray(np.asarray(a, dtype=np.float32))
    bf = lambda a: np.ascontiguousarray(a.astype(ml_dtypes.bfloat16))
    w1 = f(inputs["W1"])
    w2 = f(inputs["W2"])
    wg = f(inputs["Wg"])
    b1 = f(inputs["b1"])
    return {
        "Wg": np.ascontiguousarray(wg.reshape(DC, 128, E).transpose(1, 0, 2)),
        "bg": f(inputs["bg"]),
        "W1": bf(w1.reshape(E, DC, 128, H).transpose(2, 0, 1, 3)),
        "b1": np.ascontiguousarray(b1.reshape(E, HC, 128).transpose(2, 0, 1)),
        "W2": bf(w2.reshape(E, HC, 128, D).transpose(2, 0, 1, 3)),
        "b2": f(inputs["b2"]),
    }


def _run(inputs: dict, trace: bool = False, **kw):
    nc = _build()
    x = np.ascontiguousarray(np.asarray(inputs["x"], dtype=np.float32)).reshape(T, D)
    shared = _prep_shared(inputs)
    in_maps = [
        {"x": x[c * TC:(c + 1) * TC], **shared} for c in range(N_CORES)
    ]
    br = bass_utils.run_bass_kernel_spmd(
        nc, in_maps, core_ids=list(range(N_CORES)), trace=trace, **kw
    )
    out = np.concatenate([r["y"] for r in br.results], axis=0)
    return out.reshape(B, S, D), br


def kernel(**inputs) -> np.ndarray:
    out, _ = _run(inputs, trace=False)
    return out


# revision 17
# speedup vs baseline: 1.1746x; 1.0232x over previous
"""MoE layer (top-2 of 8 experts) — routed Trainium2 Bass kernel.

Data-parallel over tokens across 8 NeuronCores (2048 tokens/core), expert
weights replicated (bf16).  Unlike the dense baseline (all 8 experts for
every token), this kernel routes: tokens are compacted into per-expert
DRAM buffers via dma_scatter_add, each expert's FFN runs only on its own
~512 tokens (capacity 640), and the top-2 combine is a pair of
dma_gather(transpose=True) reads + one DVE add.  4x fewer matmul rows.

Per-core pipeline:
  gate   : psum_g[8,512] = Wg.T @ X.T  (exact fp32) -> top-2 mask per token
  index  : chunk-wise cumsum (Hillis-Steele on DVE) + strict-triangular
           PE matmul -> per-(token,expert) slot; slot lists for the
           scatter/gather sides built with PE transposes + a 16->128
           replication matmul.
  scatter: dma_scatter_add of bf16 x rows into zeroed xc[e*CAP + slot]
           (2 calls: rank-0 / rank-1 slots; collision-free by construction)
  ffn    : per expert: dma_start_transpose loads xc slab feature-major;
           FM (bf16) + GELU(+b1); second matmul is flipped (h-block as
           lhsT) so the output is token(slot)-major -> plain DMA to yc.
  combine: dma_gather(transpose=True) y0T/y1T feature-major by slot list,
           DVE add, then per 128-token block: mask x b2 matmul into PSUM
           followed by accumulating PE transposes; activation copy out.
"""

import os
import sys

sys.path.insert(0, "/opt/trn_rl_repo")

STOP = os.environ.get("K_STOP", "full")   # head|scatter|ffn|gather|full

from contextlib import ExitStack

import numpy as np
import ml_dtypes

import concourse.bacc as bacc
import concourse.bass as bass
import concourse.mybir as mybir
import concourse.tile as tile
from concourse import bass_utils
from concourse.masks import make_identity

N_CORES = 8
B, S, D, E, H = 4, 4096, 256, 8, 512
T = B * S                      # 16384 tokens total
TC = T // N_CORES              # 2048 tokens per core
CAP = 640                      # per-expert token capacity (mean 512, +6.5 sigma)
NCH = 16                       # 128-token chunks per core
DC = D // 128                  # 2
HC = H // 128                  # 4
TRASH = E * CAP                # overflow slot base
NXC = E * CAP + 16             # xc/yc row count (incl. trash rows)

F32 = mybir.dt.float32
F32R = mybir.dt.float32r
BF16 = mybir.dt.bfloat16
I16 = mybir.dt.int16
I32 = mybir.dt.int32
GELU = mybir.ActivationFunctionType.Gelu
IDENT = mybir.ActivationFunctionType.Identity
COPY = mybir.ActivationFunctionType.Copy
ALU = mybir.AluOpType


def _emit(tc: tile.TileContext, ctx: ExitStack, t_in: dict, scratch: dict, t_out):
    nc = tc.nc
    x_d = t_in["x"]
    wg_d, bg_d = t_in["Wg"], t_in["bg"]
    w1_d, b1_d = t_in["W1"], t_in["b1"]
    w2_d, b2_d = t_in["W2"], t_in["b2"]
    xc_d, yc_d = scratch["xc"], scratch["yc"]
    y_d = t_out

    singles = ctx.enter_context(tc.tile_pool(name="singles", bufs=1))
    xpool = ctx.enter_context(tc.tile_pool(name="xpool", bufs=1))
    hdpool = ctx.enter_context(tc.tile_pool(name="hdpool", bufs=1))
    xcpool = ctx.enter_context(tc.tile_pool(name="xcpool", bufs=2))
    hpool = ctx.enter_context(tc.tile_pool(name="hpool", bufs=2))
    ypool = ctx.enter_context(tc.tile_pool(name="ypool", bufs=4))
    opool = ctx.enter_context(tc.tile_pool(name="opool", bufs=1))
    ps_hd = ctx.enter_context(tc.tile_pool(name="ps_hd", bufs=2, space="PSUM"))
    ps_fa = ctx.enter_context(tc.tile_pool(name="ps_fa", bufs=2, space="PSUM"))
    ps_fb = ctx.enter_context(tc.tile_pool(name="ps_fb", bufs=2, space="PSUM"))
    ps_sm = ctx.enter_context(tc.tile_pool(name="ps_sm", bufs=2, space="PSUM"))

    def hd_ps():
        return ps_hd.tile([128, 512], F32, tag="hd", name="hd")

    # ---- constants ------------------------------------------------------
    ident = singles.tile([128, 128], F32)
    make_identity(nc, ident[:])
    ident_bf = singles.tile([128, 128], BF16)
    make_identity(nc, ident_bf[:])
    # strict upper-triangular as stored [k, p]: 1 iff k < p  (so that
    # matmul gives out[p, e] = sum_{k<p} rhs[k, e])
    triu = singles.tile([128, 128], F32)
    nc.gpsimd.memset(triu[:], 1.0)
    # affine iota value = p*1 + f*(-1); keep where iota < 0 (k < p reversed:
    # stored [k(part), p(free)]: value = k - p; keep (k - p) < 0 -> 1 else 0
    nc.gpsimd.affine_select(
        out=triu[:], in_=triu[:], compare_op=ALU.is_gt, fill=0.0,
        base=0, pattern=[[1, 128]], channel_multiplier=-1,
    )
    # 16->128 replication matrix: R16[k, p] = 1 iff p % 16 == k
    r16 = singles.tile([16, 128], F32)
    nc.vector.memset(r16[:], 0.0)
    for g in range(8):
        nc.vector.tensor_copy(r16[:, g * 16:(g + 1) * 16], ident[:16, :16])

    # ---- weights / biases ----------------------------------------------
    w1_sb = singles.tile([128, E, DC, H], BF16)
    w2_sb = singles.tile([128, E, HC, D], BF16)
    wg_sb = singles.tile([128, DC, E], F32)
    nc.sync.dma_start(out=wg_sb[:], in_=wg_d[:])
    b1_sb = singles.tile([128, E, HC], F32)
    nc.sync.dma_start(out=b1_sb[:], in_=b1_d[:])
    b2_sb = singles.tile([E, D], F32R)
    nc.sync.dma_start(out=b2_sb[:], in_=b2_d[:])
    bg_sb = singles.tile([E, 1], F32)
    nc.sync.dma_start(out=bg_sb[:], in_=bg_d[:, None])

    # ---- zero-fill xc + yc trash rows ----------------------------------
    zt = singles.tile([128, 10, D], BF16)
    nc.vector.memset(zt[:], 0.0)

    # ---- x loads + transposes + gate, pipelined per 512-token tile -----
    # x_t[p, g] = token g*128 + p   (for gate path)
    x_t = xpool.tile([128, NCH, D], F32, name="x_t")
    xv = x_d.rearrange("(p g) d -> p g d", p=128)
    for s in range(4):
        nc.sync.dma_start(out=x_t[:, s * 4:(s + 1) * 4, :],
                          in_=xv[:, s * 4:(s + 1) * 4, :])
    for i in range(4):                      # zero xc rows [0, 5120)
        nc.sync.dma_start(
            out=xc_d[i * 1280:(i + 1) * 1280, :].rearrange(
                "(p c) d -> p c d", c=10),
            in_=zt[:],
        )
    nc.sync.dma_start(out=xc_d[TRASH:NXC, :], in_=zt[:16, 0, :])
    nc.sync.dma_start(out=yc_d[TRASH:NXC, :], in_=zt[:16, 0, :])
    xT = xpool.tile([128, DC, TC], F32, name="xT")
    g_sb = hdpool.tile([E, TC], F32, name="g_sb")
    for s in range(4):
        for g in range(s * 4, (s + 1) * 4):
            for dc in range(DC):
                ps_t = hd_ps()
                nc.tensor.transpose(
                    out=ps_t[:, :128], in_=x_t[:, g, dc * 128:(dc + 1) * 128],
                    identity=ident[:],
                )
                nc.vector.tensor_copy(
                    xT[:, dc, g * 128:(g + 1) * 128], ps_t[:, :128])
        ps_gate = hd_ps()
        for dc in range(DC):
            nc.tensor.matmul(
                ps_gate[:E, :], wg_sb[:, dc, :], xT[:, dc, s * 512:(s + 1) * 512],
                start=(dc == 0), stop=(dc == DC - 1),
            )
        nc.scalar.activation(
            g_sb[:, s * 512:(s + 1) * 512], ps_gate[:E, :], IDENT, bias=bg_sb[:, 0:1]
        )

    # x_sc/weights issued after the gate-critical x_t load.
    # With token tau = p*16 + ch (x_t contiguous), list position i maps to
    # token i, so the scatter source needs x_sc[p, g] = x[g*128 + p].
    x_sc = xpool.tile([128, NCH, D], F32, name="x_sc")
    nc.scalar.dma_start(
        out=x_sc[:], in_=x_d.rearrange("(g p) d -> p g d", p=128)
    )
    x_bf = xpool.tile([128, NCH, D], BF16, name="x_bf")
    nc.scalar.activation(x_bf[:], x_sc[:], COPY)
    for e in range(E // 2):
        nc.scalar.dma_start(out=w1_sb[:, e], in_=w1_d[:, e])
        nc.scalar.dma_start(out=w2_sb[:, e], in_=w2_d[:, e])

    # ---- top-2 mask + slot machinery -----------------------------------
    # gtok[:, ch, :] = scores of token ch*128+p
    gtok = hdpool.tile([128, NCH, E], F32, name="gtok")
    for ch in range(NCH):
        ps_t = hd_ps()
        nc.tensor.transpose(
            out=ps_t[:, :E], in_=g_sb[:, ch * 128:(ch + 1) * 128],
            identity=ident[:E, :E],
        )
        nc.vector.tensor_copy(gtok[:, ch, :], ps_t[:, :E])
    m8 = hdpool.tile([128, NCH, 8], F32, name="m8")
    for ch in range(NCH):
        nc.vector.max(m8[:, ch, :], gtok[:, ch, :])
    mask_all = hdpool.tile([128, NCH, E], F32, name="mask_all")
    nc.vector.tensor_tensor(
        out=mask_all[:], in0=gtok[:], in1=m8[:, :, 1:2].to_broadcast([128, NCH, E]),
        op=ALU.is_ge,
    )
    sel0 = hdpool.tile([128, NCH, E], F32, name="sel0")
    nc.vector.tensor_tensor(
        out=sel0[:], in0=gtok[:], in1=m8[:, :, 0:1].to_broadcast([128, NCH, E]),
        op=ALU.is_ge,
    )
    sel1 = hdpool.tile([128, NCH, E], F32, name="sel1")
    nc.vector.tensor_tensor(out=sel1[:], in0=mask_all[:], in1=sel0[:], op=ALU.subtract)

    # inclusive cumsum over chunk axis (Hillis-Steele, ping-pong)
    cs = [hdpool.tile([128, NCH, E], F32, name=f"cs{i}") for i in range(2)]
    src = mask_all
    for i, d in enumerate((1, 2, 4, 8)):
        dst = cs[i % 2]
        nc.vector.tensor_tensor(
            out=dst[:, d:, :], in0=src[:, d:, :], in1=src[:, :NCH - d, :], op=ALU.add
        )
        nc.vector.tensor_copy(dst[:, :d, :], src[:, :d, :])
        src = dst
    csum = src                                   # [128, NCH, E] inclusive

    # partition-exclusive prefix of row totals
    rowtot = hdpool.tile([128, E], F32, name="rowtot")
    nc.vector.tensor_copy(rowtot[:], csum[:, NCH - 1, :])
    ps_pref = hd_ps()
    nc.tensor.matmul(ps_pref[:, :E], triu[:], rowtot[:], start=True, stop=True)

    # base[p, e] = pref[p, e] + e*CAP
    eoff = hdpool.tile([128, E], F32, name="eoff")
    for e in range(E):
        nc.vector.memset(eoff[:, e:e + 1], float(e * CAP))
    base_sb = hdpool.tile([128, E], F32, name="base")
    nc.vector.tensor_tensor(out=base_sb[:], in0=ps_pref[:, :E], in1=eoff[:], op=ALU.add)

    # oslot[t, e] = excl-cumsum + base ; overflow -> TRASH
    excl = hdpool.tile([128, NCH, E], F32, name="excl")
    nc.vector.tensor_tensor(out=excl[:], in0=csum[:], in1=mask_all[:], op=ALU.subtract)
    oslot = hdpool.tile([128, NCH, E], F32, name="oslot")
    nc.vector.tensor_tensor(
        out=oslot[:], in0=excl[:],
        in1=base_sb[:, None, :].to_broadcast([128, NCH, E]), op=ALU.add,
    )
    ov01 = hdpool.tile([128, NCH, E], F32, name="ov01")
    nc.vector.tensor_scalar(
        out=ov01[:], in0=excl[:], scalar1=float(CAP), scalar2=None, op0=ALU.is_ge
    )
    notover = hdpool.tile([128, NCH, E], F32, name="notover")
    nc.vector.tensor_scalar(
        out=notover[:], in0=ov01[:], scalar1=-1.0, scalar2=1.0,
        op0=ALU.mult, op1=ALU.add,
    )
    ovtrash = hdpool.tile([128, NCH, E], F32, name="ovtrash")
    nc.vector.tensor_scalar(
        out=ovtrash[:], in0=ov01[:], scalar1=float(TRASH), scalar2=None,
        op0=ALU.mult,
    )
    nc.vector.tensor_tensor(out=oslot[:], in0=oslot[:], in1=notover[:], op=ALU.mult)
    nc.vector.tensor_tensor(out=oslot[:], in0=oslot[:], in1=ovtrash[:], op=ALU.add)

    # per rank: slot list -> idx list -> scatter immediately (one 2048-desc
    # call per rank; descs proven safe up to 2048 for scatter)
    idx = []
    for r, sel in ((0, sel0), (1, sel1)):
        tmp = hdpool.tile([128, NCH, E], F32, name=f"otmp{r}")
        nc.vector.tensor_tensor(out=tmp[:], in0=sel[:], in1=oslot[:], op=ALU.mult)
        o_r = hdpool.tile([128, NCH], F32, name=f"o{r}")
        nc.vector.tensor_reduce(o_r[:], tmp[:], axis=mybir.AxisListType.X, op=ALU.add)
        ps_t = hd_ps()
        nc.tensor.transpose(out=ps_t[:16, :128], in_=o_r[:], identity=ident[:])
        t_sb = hdpool.tile([16, 128], F32, name=f"t_sb{r}")
        nc.vector.tensor_copy(t_sb[:], ps_t[:16, :128])
        ps_r = hd_ps()
        nc.tensor.matmul(ps_r[:, :128], r16[:], t_sb[:], start=True, stop=True)
        idx_r = hdpool.tile([128, 128], I16, name=f"idx{r}")
        nc.vector.tensor_copy(idx_r[:], ps_r[:, :128])
        idx.append(idx_r)
        nc.gpsimd.dma_scatter_add(
            out_ap=xc_d[:, :], in_ap=x_bf[:], idxs_ap=idx_r[:],
            num_idxs=TC, num_idxs_reg=TC, elem_size=D, queue_num=0,
        )

    # FM gather index lists: slot lists e*CAP + [0..CAP) in 16-wrap layout
    base16 = hdpool.tile([16, CAP // 16], F32, name="base16")
    for f in range(CAP // 16):
        nc.vector.memset(base16[:, f:f + 1], float(f * 16))
    # qcol[q] = q: reduce(ident16 * [16f row values]) / 16
    qtmp = hdpool.tile([16, 16], F32, name="qtmp")
    nc.vector.tensor_tensor(
        out=qtmp[:], in0=ident[:16, :16], in1=base16[:, :16], op=ALU.mult
    )
    qcol = hdpool.tile([16, 1], F32, name="qcol")
    nc.vector.tensor_reduce(qcol[:], qtmp[:], axis=mybir.AxisListType.X, op=ALU.add)
    nc.vector.tensor_scalar(
        out=qcol[:], in0=qcol[:], scalar1=1.0 / 16.0, scalar2=None, op0=ALU.mult
    )
    nc.vector.tensor_tensor(
        out=base16[:], in0=base16[:],
        in1=qcol[:].to_broadcast([16, CAP // 16]), op=ALU.add,
    )
    ps_bi = hd_ps()
    nc.tensor.matmul(ps_bi[:, :CAP // 16], r16[:], base16[:], start=True, stop=True)
    base128 = hdpool.tile([128, CAP // 16], F32, name="base128")
    nc.vector.tensor_copy(base128[:], ps_bi[:, :CAP // 16])
    idx_fm = []
    for e in range(E):
        fme = hdpool.tile([128, CAP // 16], F32, name=f"fme{e}")
        nc.vector.tensor_scalar(
            out=fme[:], in0=base128[:], scalar1=float(e * CAP), scalar2=None,
            op0=ALU.add,
        )
        ie = hdpool.tile([128, CAP // 16], I16, name=f"ifm{e}")
        nc.vector.tensor_copy(ie[:], fme[:])
        idx_fm.append(ie)

    # mt3[e, f, q] = mask[token q*128+f, e]; b2tok = mask x b2 token-major,
    # precomputed here (PE has slack) to keep the combine tail lean.
    mt3 = hdpool.tile([E, 128, NCH], F32R, name="mt3")
    for ch in range(NCH):
        ps_mt = hd_ps()
        nc.tensor.transpose(
            out=ps_mt[:E, :128], in_=mask_all[:, ch, :], identity=ident[:]
        )
        nc.vector.tensor_copy(mt3[:, :, ch], ps_mt[:E, :128])
    b2tok = hdpool.tile([128, NCH, D], F32, name="b2tok")
    for fb in range(NCH):
        ps_b2 = hd_ps()
        nc.tensor.matmul(
            ps_b2[:, :D], mt3[:, fb * 8:(fb + 1) * 8, :], b2_sb[:],
            start=True, stop=True,
        )
        nc.scalar.activation(b2tok[:, fb, :], ps_b2[:, :D], COPY)

    def _debug_out():
        outsb = opool.tile([128, NCH, D], F32, name="outsb_dbg")
        nc.vector.memset(outsb[:], 0.0)
        nc.sync.dma_start(
            out=y_d.rearrange("(g p) d -> p g d", p=128),
            in_=outsb[:],
        )

    if STOP == "head":
        _debug_out()
        return

    if "dbg_idx" in scratch:
        for r in range(2):
            nc.sync.dma_start(
                out=scratch["dbg_idx"][r * 128:(r + 1) * 128, :], in_=idx[r][:])

    if STOP == "scatter":
        _debug_out()
        return

    # ---- per-expert FFN -------------------------------------------------
    for e in range(E):
        if e == 0:
            for e2 in range(E // 2, E):
                nc.sync.dma_start(out=w1_sb[:, e2], in_=w1_d[:, e2])
                nc.sync.dma_start(out=w2_sb[:, e2], in_=w2_d[:, e2])
        xcT = xcpool.tile([128, DC, CAP], BF16, tag="xcT", name=f"xcT{e}")
        nc.gpsimd.dma_gather(
            out_ap=xcT[:], in_ap=xc_d[:, :], idxs_ap=idx_fm[e][:],
            num_idxs=CAP, num_idxs_reg=CAP, elem_size=D, transpose=True,
            queue_num=0,
        )

        h_sb = hpool.tile([128, HC, CAP], BF16, tag="h", name=f"h{e}")
        for hc in range(HC):
            ps_ha = ps_fa.tile([128, 512], F32, tag="fmA")
            ps_hb = ps_fb.tile([128, 128], F32, tag="fmB")
            for dc in range(DC):
                nc.tensor.matmul(
                    ps_ha[:], w1_sb[:, e, dc, hc * 128:(hc + 1) * 128],
                    xcT[:, dc, 0:512], start=(dc == 0), stop=(dc == DC - 1),
                )
            for dc in range(DC):
                nc.tensor.matmul(
                    ps_hb[:], w1_sb[:, e, dc, hc * 128:(hc + 1) * 128],
                    xcT[:, dc, 512:CAP], start=(dc == 0), stop=(dc == DC - 1),
                )
            nc.scalar.activation(
                h_sb[:, hc, 0:512], ps_ha[:], GELU, bias=b1_sb[:, e, hc:hc + 1]
            )
            nc.scalar.activation(
                h_sb[:, hc, 512:CAP], ps_hb[:], GELU, bias=b1_sb[:, e, hc:hc + 1]
            )
        # flipped second matmul: out[token(slot), d]
        for tb in range(CAP // 128):
            ps_o = ps_sm.tile([128, D], F32, tag="sm")
            for hc in range(HC):
                nc.tensor.matmul(
                    ps_o[:], h_sb[:, hc, tb * 128:(tb + 1) * 128],
                    w2_sb[:, e, hc, :], start=(hc == 0), stop=(hc == HC - 1),
                )
            yst = ypool.tile([128, D], BF16, tag="yst", name=f"yst{e}_{tb}")
            nc.scalar.activation(yst[:], ps_o[:], COPY)
            eng = nc.sync if (e * 5 + tb) % 2 == 0 else nc.scalar
            eng.dma_start(
                out=yc_d[e * CAP + tb * 128:e * CAP + (tb + 1) * 128, :],
                in_=yst[:],
            )

    if STOP == "ffn":
        _debug_out()
        return

    # ---- combine --------------------------------------------------------
    # non-transpose gathers land token-major: out[p, j] = yc[idx[j*128+p]]
    # = contribution of token (c*512 + j*128 + p) -> exactly the final
    # y layout; combine is two DVE adds per chunk, no PE transposes.
    outsb = opool.tile([128, NCH, D], F32, name="outsb")
    for c in range(4):
        yrc = []
        for r in range(2):
            y_rc = opool.tile([128, 4, D], BF16, tag=f"yT{r}_{c}",
                              name=f"yT{r}_{c}")
            nc.gpsimd.dma_gather(
                out_ap=y_rc[:], in_ap=yc_d[:, :],
                idxs_ap=idx[r][:, c * 32:(c + 1) * 32],
                num_idxs=512, num_idxs_reg=512, elem_size=D, transpose=False,
                queue_num=0,
            )
            yrc.append(y_rc)
        ysc = opool.tile([128, 4, D], F32, tag=f"ys{c}", name=f"ys{c}")
        nc.vector.tensor_tensor(
            out=ysc[:], in0=yrc[0][:], in1=yrc[1][:], op=ALU.add)
        nc.vector.tensor_tensor(
            out=outsb[:, c * 4:(c + 1) * 4, :], in0=ysc[:],
            in1=b2tok[:, c * 4:(c + 1) * 4, :], op=ALU.add)
    nc.sync.dma_start(
        out=y_d.rearrange("(g p) d -> p g d", p=128),
        in_=outsb[:],
    )


_CACHE = {}


def _build():
    if "nc" in _CACHE:
        return _CACHE["nc"]
    nc = bacc.Bacc("TRN2", target_bir_lowering=False)
    t_in = {
        "x": nc.dram_tensor("x", [TC, D], F32, kind="ExternalInput"),
        "Wg": nc.dram_tensor("Wg", [128, DC, E], F32, kind="ExternalInput"),
        "bg": nc.dram_tensor("bg", [E], F32, kind="ExternalInput"),
        "W1": nc.dram_tensor("W1", [128, E, DC, H], BF16, kind="ExternalInput"),
        "b1": nc.dram_tensor("b1", [128, E, HC], F32, kind="ExternalInput"),
        "W2": nc.dram_tensor("W2", [128, E, HC, D], BF16, kind="ExternalInput"),
        "b2": nc.dram_tensor("b2", [E, D], F32R, kind="ExternalInput"),
    }
    dbg = os.environ.get("K_DEBUG") == "1"
    # NOTE: scratch must be ExternalOutput, not Internal: Internal DRAM
    # compiles to a fixed NEFF address, and the 8 SPMD cores share HBM --
    # every core would scatter into the same physical buffer.  External
    # outputs get per-core runtime allocations.
    scratch = {
        "xc": nc.dram_tensor("xc", [NXC, D], BF16, kind="ExternalOutput"),
        "yc": nc.dram_tensor("yc", [NXC, D], BF16, kind="ExternalOutput"),
    }
    if dbg:
        scratch["dbg_idx"] = nc.dram_tensor(
            "dbg_idx", [256, 128], mybir.dt.int16, kind="ExternalOutput")
    y_d = nc.dram_tensor("y", [TC, D], F32, kind="ExternalOutput")
    with tile.TileContext(nc) as tc:
        with ExitStack() as ctx:
            _emit(tc, ctx, t_in, scratch, y_d)
    nc.compile()
    _CACHE["nc"] = nc
    return nc


def _prep_shared(inputs):
    f = lambda a: np.ascontiguousarray(np.asarray(a, dtype=np.float32))
    bf = lambda a: np.ascontiguousarray(a.astype(ml_dtypes.bfloat16))
    w1 = f(inputs["W1"])
    w2 = f(inputs["W2"])
    wg = f(inputs["Wg"])
    b1 = f(inputs["b1"])
    return {
        "Wg": np.ascontiguousarray(wg.reshape(DC, 128, E).transpose(1, 0, 2)),
        "bg": f(inputs["bg"]),
        "W1": bf(w1.reshape(E, DC, 128, H).transpose(2, 0, 1, 3)),
        "b1": np.ascontiguousarray(b1.reshape(E, HC, 128).transpose(2, 0, 1)),
        "W2": bf(w2.reshape(E, HC, 128, D).transpose(2, 0, 1, 3)),
        "b2": f(inputs["b2"]),
    }


def _run(inputs: dict, trace: bool = False, **kw):
    nc = _build()
    x = np.ascontiguousarray(np.asarray(inputs["x"], dtype=np.float32)).reshape(T, D)
    shared = _prep_shared(inputs)
    in_maps = [
        {"x": x[c * TC:(c + 1) * TC], **shared} for c in range(N_CORES)
    ]
    br = bass_utils.run_bass_kernel_spmd(
        nc, in_maps, core_ids=list(range(N_CORES)), trace=trace, **kw
    )
    out = np.concatenate([r["y"] for r in br.results], axis=0)
    return out.reshape(B, S, D), br


def kernel(**inputs) -> np.ndarray:
    out, _ = _run(inputs, trace=False)
    return out


# revision 18
# speedup vs baseline: 1.4251x; 1.2133x over previous
"""MoE layer (top-2 of 8 experts) Trainium2 Bass kernel.

Strategy: data-parallel over tokens across 8 NeuronCores (2048 tokens/core),
expert weights replicated (8.4 MB).  Per core, a dense all-expert FFN runs in
float32r (full PE rate); the top-2 routing mask is computed on-device in exact
fp32 and folded into the hidden activations before the second matmul, so the
expert combine happens for free in PSUM accumulation.

Dataflow per 512-token tile (feature-major layout, tokens on the free dim):
  gate   : psum_g[8,512]  = Wg.T @ X.T            (fp32, exact)
  topk   : transpose -> max8 -> threshold -> 0/1 mask -> transpose back
  ffn    : psum_h[h,512]  = W1c.T @ X.T           (f32r)
           h_sb = gelu(psum_h + b1) * maskrep     (ACT + DVE)
           psum_y[d,512] += W2c.T @ h_sb          (f32r, accumulated over e,hc)
           psum_y starts from b2 x maskT (tiny K=8 matmul)
  out    : PE-transpose Y.T -> Y, DMA out
"""

import sys

sys.path.insert(0, "/opt/trn_rl_repo")

from contextlib import ExitStack

import numpy as np

import concourse.bacc as bacc
import concourse.bass as bass
import concourse.mybir as mybir
import concourse.tile as tile
from concourse import bass_utils
from concourse.masks import make_identity

N_CORES = 8
B, S, D, E, H = 4, 4096, 256, 8, 512
T = B * S                      # 16384 tokens total
TC = T // N_CORES              # 2048 tokens per core
TILE = 512                     # tokens per tile
NTILES = TC // TILE            # 4
DC = D // 128                  # 2 d-chunks
HC = H // 128                  # 4 h-chunks

F32 = mybir.dt.float32
F32R = mybir.dt.float32r
GELU = mybir.ActivationFunctionType.Gelu
IDENT = mybir.ActivationFunctionType.Identity


def _emit(tc: tile.TileContext, ctx: ExitStack, t_in: dict, t_out):
    nc = tc.nc
    x_d, wg_d, bg_d, w1_d, b1_d, w2_d, b2_d = (
        t_in["x"], t_in["Wg"], t_in["bg"], t_in["W1"], t_in["b1"], t_in["W2"],
        t_in["b2"],
    )
    y_d = t_out

    singles = ctx.enter_context(tc.tile_pool(name="singles", bufs=1))
    xpool = ctx.enter_context(tc.tile_pool(name="xpool", bufs=2))
    xtpool = ctx.enter_context(tc.tile_pool(name="xtpool", bufs=3))
    gpool = ctx.enter_context(tc.tile_pool(name="gpool", bufs=4))
    mpool = ctx.enter_context(tc.tile_pool(name="mpool", bufs=NTILES))
    hpool = ctx.enter_context(tc.tile_pool(name="hpool", bufs=12))
    mrpool = ctx.enter_context(tc.tile_pool(name="mrpool", bufs=2))
    opool = ctx.enter_context(tc.tile_pool(name="opool", bufs=2))
    ps_h = ctx.enter_context(tc.tile_pool(name="ps_h", bufs=4, space="PSUM"))
    ps_m = ctx.enter_context(tc.tile_pool(name="ps_m", bufs=2, space="PSUM"))
    ps_y = ctx.enter_context(tc.tile_pool(name="ps_y", bufs=1, space="PSUM"))

    # ---- persistent SBUF: weights, biases, identity --------------------
    ident = singles.tile([128, 128], F32)
    make_identity(nc, ident[:])

    # W1 [E, D, H] -> per-expert [p(d%128), dc, h]; W2 -> [p(h%128), hc, d].
    # Separate tiles + alternating HWDGE rings so expert e's first matmul
    # only waits for its own 512 KB slice.
    w1_all = singles.tile([128, E, DC, H], F32R)
    w2_all = singles.tile([128, E, HC, D], F32R)
    w1_sb = [w1_all[:, e] for e in range(E)]
    w2_sb = [w2_all[:, e] for e in range(E)]
    # first half of the expert stream on the scalar ring (sync ring starts
    # with the x loads); per-expert 512KB DMAs with 4KB contiguous lines
    for e in range(E // 2):
        nc.scalar.dma_start(out=w1_all[:, e], in_=w1_d[:, e])
        nc.scalar.dma_start(out=w2_all[:, e], in_=w2_d[:, e])
    # small operands off the rings (SWDGE)
    wg_sb = singles.tile([128, DC, E], F32)
    nc.gpsimd.dma_start(out=wg_sb[:], in_=wg_d[:])
    b1_sb = singles.tile([128, E, HC], F32)
    nc.gpsimd.dma_start(out=b1_sb[:], in_=b1_d[:])
    b2_sb = singles.tile([E, D], F32R)
    nc.gpsimd.dma_start(out=b2_sb[:], in_=b2_d[:, :])
    bg_sb = singles.tile([E, 1], F32)
    nc.gpsimd.dma_start(out=bg_sb[:], in_=bg_d[:, None])
    # sel_sb[k, e*128 + m] = 1 if k == e else 0.  lhsT slice [8, 128] at
    # expert e replicates maskT row e across all 128 output partitions.
    sel_sb = singles.tile([E, E * 128], F32R)
    for e in range(E):
        nc.vector.tensor_copy(
            sel_sb[:, e * 128:(e + 1) * 128],
            ident[:E, e:e + 1].to_broadcast([E, 128]),
        )

    # ---- per-tile working set ------------------------------------------
    xt_tiles = []      # X^T  [128(d), dc, 512(tok)] per tile (exact fp32)
    xtr_tiles = []     # X^T rounded to f32r for the FFN matmuls
    mt_tiles = []      # mask^T [8, 512] per tile
    mrep_tiles = []    # mask row e replicated across partitions, per tile
    for t in range(NTILES):
        xt_tiles.append(xtpool.tile([128, DC, TILE], F32, tag="xt", name=f"xt{t}"))
        xtr_tiles.append(xtpool.tile([128, DC, TILE], F32R, tag="xtr", name=f"xtr{t}"))
        mt_tiles.append(mpool.tile([E, TILE], F32R, tag="mt", name=f"mt{t}"))
        mrep_tiles.append(mrpool.tile([128, E, TILE], F32, tag="mrep", name=f"mrep{t}"))

    # ---- phase A: x loads first (sync ring), then w2 stream, then transposes
    x_tiles = []
    for t in range(NTILES):
        t0 = t * TILE
        x_tile = xpool.tile([128, TILE // 128, D], F32, tag="x", bufs=3,
                            name=f"xld{t}")
        nc.sync.dma_start(
            out=x_tile[:],
            in_=x_d[t0:t0 + TILE, :].rearrange("(p cc) d -> p cc d", p=128),
        )
        x_tiles.append(x_tile)
    for e in range(E // 2, E):
        nc.sync.dma_start(out=w1_all[:, e], in_=w1_d[:, e])
        nc.sync.dma_start(out=w2_all[:, e], in_=w2_d[:, e])
    for t in range(NTILES):
        x_tile = x_tiles[t]
        for cc in range(TILE // 128):
            for dc in range(DC):
                ps_t = ps_m.tile([128, 128], F32, tag="pst")
                nc.tensor.transpose(
                    out=ps_t[:],
                    in_=x_tile[:, cc, dc * 128:(dc + 1) * 128],
                    identity=ident[:],
                )
                nc.vector.tensor_copy(
                    xt_tiles[t][:, dc, cc * 128:(cc + 1) * 128], ps_t[:]
                )
        nc.vector.tensor_copy(xtr_tiles[t][:], xt_tiles[t][:])

    # ---- phase B (all tiles): gate + top-2 mask ------------------------
    for t in range(NTILES):
        xt = xt_tiles[t]
        ps_g = ps_h.tile([E, TILE], F32, tag="psh")
        for dc in range(DC):
            nc.tensor.matmul(
                ps_g[:], wg_sb[:, dc, :], xt[:, dc, :],
                start=(dc == 0), stop=(dc == DC - 1),
            )
        g_sb = gpool.tile([E, TILE], F32, tag="gsb")
        nc.scalar.activation(g_sb[:], ps_g[:], IDENT, bias=bg_sb[:, 0:1])

        for cc in range(TILE // 128):
            ps_gt = ps_m.tile([128, E], F32, tag="pst")
            nc.tensor.transpose(
                out=ps_gt[:], in_=g_sb[:, cc * 128:(cc + 1) * 128],
                identity=ident[:E, :E],
            )
            gtok = gpool.tile([128, E], F32, tag="gtok")
            nc.vector.tensor_copy(gtok[:], ps_gt[:])
            m8 = gpool.tile([128, 8], F32, tag="m8")
            nc.vector.max(m8[:], gtok[:])
            mask = gpool.tile([128, E], F32, tag="mask")
            nc.vector.tensor_tensor(
                out=mask[:], in0=gtok[:],
                in1=m8[:, 1:2].to_broadcast([128, E]),
                op=mybir.AluOpType.is_ge,
            )
            ps_mt = ps_m.tile([E, 128], F32, tag="pst")
            nc.tensor.transpose(out=ps_mt[:], in_=mask[:], identity=ident[:])
            nc.vector.tensor_copy(
                mt_tiles[t][:, cc * 128:(cc + 1) * 128], ps_mt[:]
            )
        for e in range(E):
            ps_mr = ps_m.tile([128, TILE], F32, tag="pst")
            nc.tensor.matmul(
                ps_mr[:], sel_sb[:, e * 128:(e + 1) * 128],
                mt_tiles[t][:, :],
                start=True, stop=True,
            )
            nc.vector.tensor_copy(mrep_tiles[t][:, e, :], ps_mr[:])

    # ---- phase C: software-pipelined dense masked FFN ------------------
    # PE executes its stream in order, so the second matmuls of step s-1
    # are emitted AFTER the first matmuls of step s: by the time PE reaches
    # SM(s-1), the gelu+mask chain for its h tiles has had a full step to
    # drain, and PE never stalls on ACT/DVE latency.
    NSTEP = NTILES * E
    h_live = {}

    def emit_fm(t, e):
        xtr = xtr_tiles[t]
        mrep = mrep_tiles[t]
        tiles = []
        for hc in range(HC):
            ps_hh = ps_h.tile([128, TILE], F32, tag="psh",
                              name=f"psh{t}_{e}_{hc}")
            for dc in range(DC):
                nc.tensor.matmul(
                    ps_hh[:],
                    w1_sb[e][:, dc, hc * 128:(hc + 1) * 128],
                    xtr[:, dc, :],
                    start=(dc == 0), stop=(dc == DC - 1),
                )
            h_sb = hpool.tile([128, TILE], F32R, tag="h", name=f"h{t}_{e}_{hc}")
            nc.scalar.activation(
                h_sb[:], ps_hh[:], GELU, bias=b1_sb[:, e, hc:hc + 1]
            )
            eng = nc.vector if hc % 2 == 0 else nc.gpsimd
            eng.tensor_mul(h_sb[:], h_sb[:], mrep[:, e, :])
            tiles.append(h_sb)
        h_live[(t, e)] = tiles

    def emit_b2(t):
        for dc in range(DC):
            nc.tensor.matmul(
                psum_y[t][:, dc, :],
                b2_sb[:, dc * 128:(dc + 1) * 128],
                mt_tiles[t][:, :],
                start=True, stop=False, skip_group_check=True,
            )

    def emit_sm(t, e):
        tiles = h_live.pop((t, e))
        for hc in range(HC):
            for dc in range(DC):
                nc.tensor.matmul(
                    psum_y[t][:, dc, :],
                    w2_sb[e][:, hc, dc * 128:(dc + 1) * 128],
                    tiles[hc][:],
                    start=False,
                    stop=(e == E - 1 and hc == HC - 1 and dc == DC - 1),
                    skip_group_check=True,
                )

    def emit_ycopy(t):
        ysb = opool.tile([128, DC, TILE], F32, tag="ysb", name=f"ysb{t}")
        nc.vector.tensor_copy(ysb[:, 0, :], psum_y[t][:, 0, :])
        nc.scalar.activation(
            ysb[:, 1, :], psum_y[t][:, 1, :],
            mybir.ActivationFunctionType.Copy,
        )
        y_live[t] = ysb

    def emit_out(t):
        t0 = t * TILE
        ysb = y_live.pop(t)
        yt_sb = opool.tile([128, TILE // 128, D], F32, tag="ytsb",
                           name=f"ytsb{t}")
        for cc in range(TILE // 128):
            for dc in range(DC):
                ps_t = ps_m.tile([128, 128], F32, tag="pst",
                                 name=f"pso{t}_{cc}_{dc}")
                nc.tensor.transpose(
                    out=ps_t[:],
                    in_=ysb[:, dc, cc * 128:(cc + 1) * 128],
                    identity=ident[:],
                )
                dst = yt_sb[:, cc, dc * 128:(dc + 1) * 128]
                if (cc * DC + dc) % 2 == 0:
                    nc.vector.tensor_copy(dst, ps_t[:])
                else:
                    nc.scalar.activation(
                        dst, ps_t[:], mybir.ActivationFunctionType.Copy
                    )
        nc.sync.dma_start(
            out=y_d[t0:t0 + TILE, :].rearrange("(p cc) d -> p cc d", p=128),
            in_=yt_sb[:],
        )

    psum_y = {}
    y_live = {}
    for t in range(NTILES):
        psum_y[t] = ps_y.tile([128, DC, TILE], F32, tag="psy",
                              name=f"psy{t}")

    LAG = 2
    for s in range(NSTEP + LAG + 1):
        if s < NSTEP:
            t, e = divmod(s, E)
            emit_fm(t, e)
            if e == LAG:
                emit_b2(t)
        if s >= LAG and s - LAG < NSTEP:
            tp, ep = divmod(s - LAG, E)
            emit_sm(tp, ep)
            if ep == E - 1:
                emit_ycopy(tp)
        if s >= LAG + 1 and s - LAG - 1 < NSTEP:
            tq, eq = divmod(s - LAG - 1, E)
            if eq == E - 1:
                emit_out(tq)

_CACHE = {}

def _build():
    if "nc" in _CACHE:
        return _CACHE["nc"]
    nc = bacc.Bacc("TRN2", target_bir_lowering=False)
    t_in = {
        "x": nc.dram_tensor("x", [TC, D], F32, kind="ExternalInput"),
        "Wg": nc.dram_tensor("Wg", [128, DC, E], F32, kind="ExternalInput"),
        "bg": nc.dram_tensor("bg", [E], F32, kind="ExternalInput"),
        "W1": nc.dram_tensor("W1", [128, E, DC, H], F32R, kind="ExternalInput"),
        "b1": nc.dram_tensor("b1", [128, E, HC], F32, kind="ExternalInput"),
        "W2": nc.dram_tensor("W2", [128, E, HC, D], F32R, kind="ExternalInput"),
        "b2": nc.dram_tensor("b2", [E, D], F32R, kind="ExternalInput"),
    }
    y_d = nc.dram_tensor("y", [TC, D], F32, kind="ExternalOutput")
    with tile.TileContext(nc) as tc:
        with ExitStack() as ctx:
            _emit(tc, ctx, t_in, y_d)
    nc.compile()
    _CACHE["nc"] = nc
    return nc


def _run(inputs: dict, trace: bool = False, **kw):
    nc = _build()
    f = lambda a: np.ascontiguousarray(np.asarray(a, dtype=np.float32))
    x = f(inputs["x"]).reshape(T, D)
    w1 = f(inputs["W1"])
    w2 = f(inputs["W2"])
    wg = f(inputs["Wg"])
    b1 = f(inputs["b1"])
    shared = {
        "Wg": np.ascontiguousarray(wg.reshape(DC, 128, E).transpose(1, 0, 2)),
        "bg": f(inputs["bg"]),
        "W1": np.ascontiguousarray(
            w1.reshape(E, DC, 128, H).transpose(2, 0, 1, 3)),
        "b1": np.ascontiguousarray(
            b1.reshape(E, HC, 128).transpose(2, 0, 1)),
        "W2": np.ascontiguousarray(
            w2.reshape(E, HC, 128, D).transpose(2, 0, 1, 3)),
        "b2": f(inputs["b2"]),
    }
    in_maps = [
        {"x": x[c * TC:(c + 1) * TC], **shared} for c in range(N_CORES)
    ]
    br = bass_utils.run_bass_kernel_spmd(
        nc, in_maps, core_ids=list(range(N_CORES)), trace=trace, **kw
    )
    out = np.concatenate([r["y"] for r in br.results], axis=0)
    return out.reshape(B, S, D), br


def kernel(**inputs) -> np.ndarray:
    out, _ = _run(inputs, trace=False)
    return out



# revision 19
# speedup vs baseline: 1.4816x; 1.0396x over previous
"""MoE layer (top-2 of 8 experts) Trainium2 Bass kernel.

Strategy: data-parallel over tokens across 8 NeuronCores (2048 tokens/core),
expert weights replicated (8.4 MB).  Per core, a dense all-expert FFN runs in
float32r (full PE rate); the top-2 routing mask is computed on-device in exact
fp32 and folded into the hidden activations before the second matmul, so the
expert combine happens for free in PSUM accumulation.

Dataflow per 512-token tile (feature-major layout, tokens on the free dim):
  gate   : psum_g[8,512]  = Wg.T @ X.T            (fp32, exact)
  topk   : transpose -> max8 -> threshold -> 0/1 mask -> transpose back
  ffn    : psum_h[h,512]  = W1c.T @ X.T           (f32r)
           h_sb = gelu(psum_h + b1) * maskrep     (ACT + DVE)
           psum_y[d,512] += W2c.T @ h_sb          (f32r, accumulated over e,hc)
           psum_y starts from b2 x maskT (tiny K=8 matmul)
  out    : PE-transpose Y.T -> Y, DMA out
"""

import sys

sys.path.insert(0, "/opt/trn_rl_repo")

from contextlib import ExitStack

import numpy as np

import concourse.bacc as bacc
import concourse.bass as bass
import concourse.mybir as mybir
import concourse.tile as tile
from concourse import bass_utils
from concourse.masks import make_identity

N_CORES = 8
B, S, D, E, H = 4, 4096, 256, 8, 512
T = B * S                      # 16384 tokens total
TC = T // N_CORES              # 2048 tokens per core
TILE = 512                     # tokens per tile
NTILES = TC // TILE            # 4
DC = D // 128                  # 2 d-chunks
HC = H // 128                  # 4 h-chunks

F32 = mybir.dt.float32
F32R = mybir.dt.float32r
BF16 = mybir.dt.bfloat16
GELU = mybir.ActivationFunctionType.Gelu
IDENT = mybir.ActivationFunctionType.Identity


def _emit(tc: tile.TileContext, ctx: ExitStack, t_in: dict, t_out):
    nc = tc.nc
    x_d, wg_d, bg_d, w1_d, b1_d, w2_d, b2_d = (
        t_in["x"], t_in["Wg"], t_in["bg"], t_in["W1"], t_in["b1"], t_in["W2"],
        t_in["b2"],
    )
    y_d = t_out

    singles = ctx.enter_context(tc.tile_pool(name="singles", bufs=1))
    xpool = ctx.enter_context(tc.tile_pool(name="xpool", bufs=2))
    xtpool = ctx.enter_context(tc.tile_pool(name="xtpool", bufs=3))
    gpool = ctx.enter_context(tc.tile_pool(name="gpool", bufs=4))
    mpool = ctx.enter_context(tc.tile_pool(name="mpool", bufs=NTILES))
    hpool = ctx.enter_context(tc.tile_pool(name="hpool", bufs=12))
    mrpool = ctx.enter_context(tc.tile_pool(name="mrpool", bufs=2))
    opool = ctx.enter_context(tc.tile_pool(name="opool", bufs=2))
    ps_h = ctx.enter_context(tc.tile_pool(name="ps_h", bufs=4, space="PSUM"))
    ps_m = ctx.enter_context(tc.tile_pool(name="ps_m", bufs=2, space="PSUM"))
    ps_y = ctx.enter_context(tc.tile_pool(name="ps_y", bufs=1, space="PSUM"))

    # ---- persistent SBUF: weights, biases, identity --------------------
    ident = singles.tile([128, 128], F32)
    make_identity(nc, ident[:])
    ident_bf = singles.tile([128, 128], BF16)
    make_identity(nc, ident_bf[:])

    # W1 [E, D, H] -> per-expert [p(d%128), dc, h]; W2 -> [p(h%128), hc, d].
    # Separate tiles + alternating HWDGE rings so expert e's first matmul
    # only waits for its own 512 KB slice.
    w1_all = singles.tile([128, E, DC, H], F32R)
    w2_all = singles.tile([128, E, HC, D], F32R)
    w1_sb = [w1_all[:, e] for e in range(E)]
    w2_sb = [w2_all[:, e] for e in range(E)]
    # first half of the expert stream on the scalar ring (sync ring starts
    # with the x loads); per-expert 512KB DMAs with 4KB contiguous lines
    for e in range(E // 2):
        nc.scalar.dma_start(out=w1_all[:, e], in_=w1_d[:, e])
        nc.scalar.dma_start(out=w2_all[:, e], in_=w2_d[:, e])
    # small operands off the rings (SWDGE)
    wg_sb = singles.tile([128, DC, E], F32)
    nc.gpsimd.dma_start(out=wg_sb[:], in_=wg_d[:])
    b1_sb = singles.tile([128, E, HC], F32)
    nc.gpsimd.dma_start(out=b1_sb[:], in_=b1_d[:])
    b2_sb = singles.tile([E, D], F32R)
    nc.gpsimd.dma_start(out=b2_sb[:], in_=b2_d[:, :])
    bg_sb = singles.tile([E, 1], F32)
    nc.gpsimd.dma_start(out=bg_sb[:], in_=bg_d[:, None])
    # sel_sb[k, e*128 + m] = 1 if k == e else 0.  lhsT slice [8, 128] at
    # expert e replicates maskT row e across all 128 output partitions.
    sel_sb = singles.tile([E, E * 128], F32R)
    for e in range(E):
        nc.vector.tensor_copy(
            sel_sb[:, e * 128:(e + 1) * 128],
            ident[:E, e:e + 1].to_broadcast([E, 128]),
        )

    # ---- per-tile working set ------------------------------------------
    xt_tiles = []      # X^T  [128(d), dc, 512(tok)] per tile (exact fp32)
    xtr_tiles = []     # X^T rounded to f32r for the FFN matmuls
    mt_tiles = []      # mask^T [8, 512] per tile
    mrep_tiles = []    # mask row e replicated across partitions, per tile
    for t in range(NTILES):
        xt_tiles.append(xtpool.tile([128, DC, TILE], F32, tag="xt", name=f"xt{t}"))
        xtr_tiles.append(xtpool.tile([128, DC, TILE], F32R, tag="xtr", name=f"xtr{t}"))
        mt_tiles.append(mpool.tile([E, TILE], F32R, tag="mt", name=f"mt{t}"))
        mrep_tiles.append(mrpool.tile([128, E, TILE], F32, tag="mrep", name=f"mrep{t}"))

    # ---- phase A: x loads first (sync ring), then w2 stream, then transposes
    x_tiles = []
    for t in range(NTILES):
        t0 = t * TILE
        x_tile = xpool.tile([128, TILE // 128, D], F32, tag="x", bufs=3,
                            name=f"xld{t}")
        nc.sync.dma_start(
            out=x_tile[:],
            in_=x_d[t0:t0 + TILE, :].rearrange("(p cc) d -> p cc d", p=128),
        )
        x_tiles.append(x_tile)
    for e in range(E // 2, E):
        nc.sync.dma_start(out=w1_all[:, e], in_=w1_d[:, e])
        nc.sync.dma_start(out=w2_all[:, e], in_=w2_d[:, e])
    for t in range(NTILES):
        x_tile = x_tiles[t]
        for cc in range(TILE // 128):
            for dc in range(DC):
                ps_t = ps_m.tile([128, 128], F32, tag="pst")
                nc.tensor.transpose(
                    out=ps_t[:],
                    in_=x_tile[:, cc, dc * 128:(dc + 1) * 128],
                    identity=ident[:],
                )
                nc.vector.tensor_copy(
                    xt_tiles[t][:, dc, cc * 128:(cc + 1) * 128], ps_t[:]
                )
        nc.vector.tensor_copy(xtr_tiles[t][:], xt_tiles[t][:])

    # ---- phase B (all tiles): gate + top-2 mask ------------------------
    for t in range(NTILES):
        xt = xt_tiles[t]
        ps_g = ps_h.tile([E, TILE], F32, tag="psh")
        for dc in range(DC):
            nc.tensor.matmul(
                ps_g[:], wg_sb[:, dc, :], xt[:, dc, :],
                start=(dc == 0), stop=(dc == DC - 1),
            )
        g_sb = gpool.tile([E, TILE], F32, tag="gsb")
        nc.scalar.activation(g_sb[:], ps_g[:], IDENT, bias=bg_sb[:, 0:1])

        for cc in range(TILE // 128):
            ps_gt = ps_m.tile([128, E], F32, tag="pst")
            nc.tensor.transpose(
                out=ps_gt[:], in_=g_sb[:, cc * 128:(cc + 1) * 128],
                identity=ident[:E, :E],
            )
            gtok = gpool.tile([128, E], F32, tag="gtok")
            nc.vector.tensor_copy(gtok[:], ps_gt[:])
            m8 = gpool.tile([128, 8], F32, tag="m8")
            nc.vector.max(m8[:], gtok[:])
            mask = gpool.tile([128, E], F32, tag="mask")
            nc.vector.tensor_tensor(
                out=mask[:], in0=gtok[:],
                in1=m8[:, 1:2].to_broadcast([128, E]),
                op=mybir.AluOpType.is_ge,
            )
            ps_mt = ps_m.tile([E, 128], F32, tag="pst")
            nc.tensor.transpose(out=ps_mt[:], in_=mask[:], identity=ident[:])
            nc.vector.tensor_copy(
                mt_tiles[t][:, cc * 128:(cc + 1) * 128], ps_mt[:]
            )
        for e in range(E):
            ps_mr = ps_m.tile([128, TILE], F32, tag="pst")
            nc.tensor.matmul(
                ps_mr[:], sel_sb[:, e * 128:(e + 1) * 128],
                mt_tiles[t][:, :],
                start=True, stop=True,
            )
            nc.vector.tensor_copy(mrep_tiles[t][:, e, :], ps_mr[:])

    # ---- phase C: software-pipelined dense masked FFN ------------------
    # PE executes its stream in order, so the second matmuls of step s-1
    # are emitted AFTER the first matmuls of step s: by the time PE reaches
    # SM(s-1), the gelu+mask chain for its h tiles has had a full step to
    # drain, and PE never stalls on ACT/DVE latency.
    NSTEP = NTILES * E
    h_live = {}

    def emit_fm(t, e):
        xtr = xtr_tiles[t]
        mrep = mrep_tiles[t]
        tiles = []
        for hc in range(HC):
            ps_hh = ps_h.tile([128, TILE], F32, tag="psh",
                              name=f"psh{t}_{e}_{hc}")
            for dc in range(DC):
                nc.tensor.matmul(
                    ps_hh[:],
                    w1_sb[e][:, dc, hc * 128:(hc + 1) * 128],
                    xtr[:, dc, :],
                    start=(dc == 0), stop=(dc == DC - 1),
                )
            h_sb = hpool.tile([128, TILE], F32R, tag="h", name=f"h{t}_{e}_{hc}")
            nc.scalar.activation(
                h_sb[:], ps_hh[:], GELU, bias=b1_sb[:, e, hc:hc + 1]
            )
            eng = nc.vector if hc % 2 == 0 else nc.gpsimd
            eng.tensor_mul(h_sb[:], h_sb[:], mrep[:, e, :])
            tiles.append(h_sb)
        h_live[(t, e)] = tiles

    def emit_b2(t):
        for dc in range(DC):
            nc.tensor.matmul(
                psum_y[t][:, dc, :],
                b2_sb[:, dc * 128:(dc + 1) * 128],
                mt_tiles[t][:, :],
                start=True, stop=False, skip_group_check=True,
            )

    def emit_sm(t, e):
        tiles = h_live.pop((t, e))
        for hc in range(HC):
            for dc in range(DC):
                nc.tensor.matmul(
                    psum_y[t][:, dc, :],
                    w2_sb[e][:, hc, dc * 128:(dc + 1) * 128],
                    tiles[hc][:],
                    start=False,
                    stop=(e == E - 1 and hc == HC - 1 and dc == DC - 1),
                    skip_group_check=True,
                )

    def emit_ycopy(t):
        ysb = opool.tile([128, DC, TILE], BF16, tag="ysb", name=f"ysb{t}")
        nc.vector.tensor_copy(ysb[:, 0, :], psum_y[t][:, 0, :])
        nc.scalar.activation(
            ysb[:, 1, :], psum_y[t][:, 1, :],
            mybir.ActivationFunctionType.Copy,
        )
        y_live[t] = ysb

    def emit_out(t):
        t0 = t * TILE
        ysb = y_live.pop(t)
        yt_sb = opool.tile([128, TILE // 128, D], F32, tag="ytsb",
                           name=f"ytsb{t}")
        for cc in range(TILE // 128):
            for dc in range(DC):
                ps_t = ps_m.tile([128, 128], BF16, tag="pst",
                                 name=f"pso{t}_{cc}_{dc}")
                nc.tensor.transpose(
                    out=ps_t[:],
                    in_=ysb[:, dc, cc * 128:(cc + 1) * 128],
                    identity=ident_bf[:],
                )
                dst = yt_sb[:, cc, dc * 128:(dc + 1) * 128]
                if (cc * DC + dc) % 2 == 0:
                    nc.vector.tensor_copy(dst, ps_t[:])
                else:
                    nc.scalar.activation(
                        dst, ps_t[:], mybir.ActivationFunctionType.Copy
                    )
        nc.sync.dma_start(
            out=y_d[t0:t0 + TILE, :].rearrange("(p cc) d -> p cc d", p=128),
            in_=yt_sb[:],
        )

    psum_y = {}
    y_live = {}
    for t in range(NTILES):
        psum_y[t] = ps_y.tile([128, DC, TILE], F32, tag="psy",
                              name=f"psy{t}")

    LAG = 2
    for s in range(NSTEP + LAG + 1):
        if s < NSTEP:
            t, e = divmod(s, E)
            emit_fm(t, e)
            if e == LAG:
                emit_b2(t)
        if s >= LAG and s - LAG < NSTEP:
            tp, ep = divmod(s - LAG, E)
            emit_sm(tp, ep)
            if ep == E - 1:
                emit_ycopy(tp)
        if s >= LAG + 1 and s - LAG - 1 < NSTEP:
            tq, eq = divmod(s - LAG - 1, E)
            if eq == E - 1:
                emit_out(tq)

_CACHE = {}

def _build():
    if "nc" in _CACHE:
        return _CACHE["nc"]
    nc = bacc.Bacc("TRN2", target_bir_lowering=False)
    t_in = {
        "x": nc.dram_tensor("x", [TC, D], F32, kind="ExternalInput"),
        "Wg": nc.dram_tensor("Wg", [128, DC, E], F32, kind="ExternalInput"),
        "bg": nc.dram_tensor("bg", [E], F32, kind="ExternalInput"),
        "W1": nc.dram_tensor("W1", [128, E, DC, H], F32R, kind="ExternalInput"),
        "b1": nc.dram_tensor("b1", [128, E, HC], F32, kind="ExternalInput"),
        "W2": nc.dram_tensor("W2", [128, E, HC, D], F32R, kind="ExternalInput"),
        "b2": nc.dram_tensor("b2", [E, D], F32R, kind="ExternalInput"),
    }
    y_d = nc.dram_tensor("y", [TC, D], F32, kind="ExternalOutput")
    with tile.TileContext(nc) as tc:
        with ExitStack() as ctx:
            _emit(tc, ctx, t_in, y_d)
    nc.compile()
    _CACHE["nc"] = nc
    return nc


def _run(inputs: dict, trace: bool = False, **kw):
    nc = _build()
    f = lambda a: np.ascontiguousarray(np.asarray(a, dtype=np.float32))
    x = f(inputs["x"]).reshape(T, D)
    w1 = f(inputs["W1"])
    w2 = f(inputs["W2"])
    wg = f(inputs["Wg"])
    b1 = f(inputs["b1"])
    shared = {
        "Wg": np.ascontiguousarray(wg.reshape(DC, 128, E).transpose(1, 0, 2)),
        "bg": f(inputs["bg"]),
        "W1": np.ascontiguousarray(
            w1.reshape(E, DC, 128, H).transpose(2, 0, 1, 3)),
        "b1": np.ascontiguousarray(
            b1.reshape(E, HC, 128).transpose(2, 0, 1)),
        "W2": np.ascontiguousarray(
            w2.reshape(E, HC, 128, D).transpose(2, 0, 1, 3)),
        "b2": f(inputs["b2"]),
    }
    in_maps = [
        {"x": x[c * TC:(c + 1) * TC], **shared} for c in range(N_CORES)
    ]
    br = bass_utils.run_bass_kernel_spmd(
        nc, in_maps, core_ids=list(range(N_CORES)), trace=trace, **kw
    )
    out = np.concatenate([r["y"] for r in br.results], axis=0)
    return out.reshape(B, S, D), br


def kernel(**inputs) -> np.ndarray:
    out, _ = _run(inputs, trace=False)
    return out

